# revision 20
# baseline (speedup 1.0000x reference)
"""Fused multi-head attention + output projection for Trainium2 (Bass/Tile).

Problem: B=4, N=2048, C=768, H=12 heads x D=64.
  qkv = x @ W_qkv + b_qkv ; q,k,v per head ; attn = softmax(q k^T / sqrt(D))
  attn_out = (attn @ v) merged ; out = attn_out @ W_proj + b_proj
  returns (out, attn_out)

Sharding over 8 NeuronCores: core c = (b, g) with b = batch (4), g = head
group (2 groups of 6 heads).  Data-parallel over batch, tensor-parallel over
heads: W_qkv columns / W_proj rows are split per group; the N x N attention
matrix stays core-local.  Host only slices inputs and, on gather, transposes
the (feature-major) outputs and sums the two W_proj partial products per
batch.

Per-core device algorithm (all layouts feature-major "T" = [features, n]):
  xT = transpose(x_b)                       (PE transposes via identity)
  qkT[f, n] = W_qk^T x (+bias, q pre-scaled on host)      fp32r matmuls
  v[n, f] (+bias via ones-row matmul), stored bf16 augmented with a ones
      column per head -> PV matmul also yields softmax row-sums.
  Per head: S^T[k, q] = kT^T qT (no max subtraction needed: |S| <= ~6),
      P^T = exp(S^T) on ScalarE straight out of PSUM (bf16),
      ctx^T[d, q] (+rowsum row) = [v|1]^T @ P^T, normalize by 1/rowsum.
  out^T = W_proj^T ctx^T (+b_proj on group-0 cores only, via zeroed input).

All phases share one 6-bank PSUM "ring" of [128,512] units (plus 2 banks of
PV accumulators), so no PSUM pool boundary serializes phase transitions.
"""

import os
import numpy as np
import ml_dtypes
from contextlib import ExitStack

import concourse.bass as bass
import concourse.tile as tile
import concourse.mybir as mybir
from concourse import bacc
import concourse.bass_utils as _bass_utils
from concourse.bass_utils import run_bass_kernel_spmd

# walrus is invoked with --enable-ldw-opt=false by default, which forces a
# serial LDWEIGHTS before every MATMUL (~250us of PE time for this kernel).
_orig_run_command = _bass_utils.run_command


def _run_command_ldw(argv, **kw):
    argv = ["--enable-ldw-opt=true" if a == "--enable-ldw-opt=false" else a
            for a in argv]
    return _orig_run_command(argv, **kw)


# NOTE: tried --enable-ldw-opt=true: walrus rejects it for fp32/fp32r
# weights ("InstLdweights is not compatible with LDW optimization").
ENABLE_LDW_OPT = bool(os.environ.get("K_LDW_OPT"))
if ENABLE_LDW_OPT and _bass_utils.run_command is _orig_run_command:
    _bass_utils.run_command = _run_command_ldw

N_CORES = 8
B, N, C = 4, 2048, 768
H, D = 12, 64
G = 2                # head groups (tensor-parallel)
HL = H // G          # heads per core
CL = HL * D          # local feature width (384)
SCALE = D ** -0.5
NT = N // 128        # 16 row tiles
CC = C // 128        # 6 contraction chunks
QC = N // 512        # 4 q chunks of 512
KT = N // 128        # 16 k tiles
FQK = 2 * CL // 128  # 6 feature tiles for q|k

F32 = mybir.dt.float32
F32R = mybir.dt.float32r
BF16 = mybir.dt.bfloat16
I16 = mybir.dt.int16

# Schraudolph fast-exp in bf16 bit space: bf16_bits(exp(x)) ~ x*FE_A + FE_B.
# FE_B calibrated on HW (trunc semantics) for zero-mean relative error.
FE_A = 2.0 ** 7 / float(np.log(2.0))
FE_B = 16256.5 - 7.88
AF = mybir.ActivationFunctionType
ALU = mybir.AluOpType

_CACHE = {}


class Ring:
    """Rotating [128, 512] PSUM units inside one 6-bank tensor."""

    def __init__(self, t, n_units=6):
        self.t = t
        self.n = n_units
        self.pos = 0

    def unit(self, width=512):
        p = self.pos % self.n
        self.pos += 1
        return self.t[:, p * 512:p * 512 + width], p

    def align(self, m):
        self.pos = ((self.pos + m - 1) // m) * m


def _build_nc(repeat=1):
    nc = bacc.Bacc("TRN2", target_bir_lowering=False, debug=False,
                   num_devices=N_CORES)
    xt_d = nc.dram_tensor("xT", [C, N], BF16, kind="ExternalInput").ap()
    wqk_d = nc.dram_tensor("w_qk", [C, 2 * CL], BF16, kind="ExternalInput").ap()
    wv_d = nc.dram_tensor("w_v", [C, CL], BF16, kind="ExternalInput").ap()
    wp_d = nc.dram_tensor("w_p", [CL, C], BF16, kind="ExternalInput").ap()
    bqk_d = nc.dram_tensor("b_qk", [128, FQK], F32, kind="ExternalInput").ap()
    bv_d = nc.dram_tensor("b_v", [1, CL], BF16, kind="ExternalInput").ap()
    bp_d = nc.dram_tensor("b_p", [128, C // 128], F32, kind="ExternalInput").ap()
    onesc_d = nc.dram_tensor("ones_col", [1, 128], BF16, kind="ExternalInput").ap()
    aot_d = nc.dram_tensor("attn_out_t", [CL, N], BF16, kind="ExternalOutput").ap()
    out_d = nc.dram_tensor("out_t", [C, N], F32R, kind="ExternalOutput").ap()
    DEBUG = bool(os.environ.get("K_DEBUG"))
    if DEBUG:
        dbg_rs = nc.dram_tensor("dbg_rs", [1, 512], F32, kind="ExternalOutput").ap()
        dbg_rA = nc.dram_tensor("dbg_rA", [1, 512], F32, kind="ExternalOutput").ap()
        dbg_bc = nc.dram_tensor("dbg_bc", [64, 512], F32, kind="ExternalOutput").ap()

    with tile.TileContext(nc) as tc:
      for _rep in range(repeat):
        with ExitStack() as top:
            const_pool = top.enter_context(tc.tile_pool(name="const", bufs=1))
            bias_pool = top.enter_context(tc.tile_pool(name="bias", bufs=3))
            qkT_pool = top.enter_context(tc.tile_pool(name="qkT", bufs=FQK))
            vaug_pool = top.enter_context(tc.tile_pool(name="vaug", bufs=NT))
            wp_pool = top.enter_context(tc.tile_pool(name="wp", bufs=3))
            ring_pool = top.enter_context(
                tc.tile_pool(name="ring", bufs=1, space="PSUM"))

            ring = Ring(ring_pool.tile([128, 3072], F32, tag="ring", name="ringt"), n_units=6)

            # x arrives pre-transposed from the host: plain parallel DMA
            # loads instead of the serialized transpose-xbar path.
            xT_pool_o = top.enter_context(tc.tile_pool(name="xT", bufs=CC))
            xT = [xT_pool_o.tile([128, N], BF16, tag="xT", name=f"xTt{i}")
                  for i in range(CC)]
            for cc in range(CC):
                nc.sync.dma_start(xT[cc][:], xt_d[cc * 128:(cc + 1) * 128, :])

            ones_col = const_pool.tile([1, 128], BF16, tag="ones")
            nc.sync.dma_start(ones_col[:], onesc_d[:])

            b_qk = bias_pool.tile([128, FQK], F32, tag="bqk")
            nc.sync.dma_start(b_qk[:], bqk_d[:])
            b_p = bias_pool.tile([128, C // 128], F32, tag="bp")
            nc.sync.dma_start(b_p[:], bp_d[:])
            b_v = bias_pool.tile([1, CL], BF16, tag="bv")
            nc.sync.dma_start(b_v[:], bv_d[:])

            wp = []
            for i in range(3):
                t = wp_pool.tile([128, C], BF16, tag="wp", name=f"wp{i}")
                nc.sync.dma_start(t[:], wp_d[i * 128:(i + 1) * 128, :])
                wp.append(t)

            qkT = [qkT_pool.tile([128, N], BF16, tag="qkT", name=f"qkT{i}")
                   for i in range(FQK)]
            vaug = [vaug_pool.tile([128, HL * 65], BF16, tag="vaug",
                                   name=f"vaug{i}") for i in range(NT)]
            # ------------- Phases A (xT), B (qkT), C (v) -------------
            with ExitStack() as s1:
                wv_pool = s1.enter_context(tc.tile_pool(name="wv", bufs=CC))

                wv = []
                for cc in range(CC):
                    t = wv_pool.tile([128, CL], BF16, tag="wv", name=f"wv{cc}")
                    nc.sync.dma_start(t[:], wv_d[cc * 128:(cc + 1) * 128, :])
                    wv.append(t)

                with ExitStack() as s1a:
                    wqk_pool = s1a.enter_context(
                        tc.tile_pool(name="wqk", bufs=CC))
                    wqk = []
                    for cc in range(CC):
                        t = wqk_pool.tile([128, 2 * CL], BF16, tag="wqk",
                                          name=f"wqk{cc}")
                        nc.sync.dma_start(t[:], wqk_d[cc * 128:(cc + 1) * 128, :])
                        wqk.append(t)


                    # B: qkT = W_qk^T @ x^T (+ per-partition bias on evac)
                    for ft in range(FQK):
                        for qc in range(QC):
                            ps, _ = ring.unit()
                            for cc in range(CC):
                                nc.tensor.matmul(
                                    ps[:], wqk[cc][:, ft * 128:(ft + 1) * 128],
                                    xT[cc][:, qc * 512:(qc + 1) * 512],
                                    start=(cc == 0), stop=(cc == CC - 1))
                            if qc % 2 == 0:
                                nc.vector.tensor_scalar_add(
                                    qkT[ft][:, qc * 512:(qc + 1) * 512], ps[:],
                                    b_qk[:, ft:ft + 1])
                            else:
                                nc.scalar.activation(
                                    qkT[ft][:, qc * 512:(qc + 1) * 512], ps[:],
                                    AF.Identity, bias=b_qk[:, ft:ft + 1])

                # C: v natural (+bias via ones-row), ones col per head
                for nt in range(NT):
                    ps, _ = ring.unit(width=CL)
                    for cc in range(CC):
                        nc.tensor.matmul(
                            ps[:], xT[cc][:, nt * 128:(nt + 1) * 128], wv[cc][:],
                            start=(cc == 0), stop=False)
                    nc.tensor.matmul(ps[:], ones_col[:], b_v[:],
                                     start=False, stop=True)
                    va3 = vaug[nt][:].rearrange("p (h e) -> p h e", e=65)
                    if nt % 2 == 0:
                        nc.vector.tensor_copy(
                            va3[:, :, 0:64],
                            ps[:].rearrange("p (h e) -> p h e", e=64))
                    else:
                        nc.scalar.activation(
                            va3[:, :, 0:64],
                            ps[:].rearrange("p (h e) -> p h e", e=64),
                            AF.Identity)
                    nc.vector.memset(va3[:, :, 64:65], 1.0)

            # ---------------- Phases D (attention) + E (proj) ----------------
            # qc-major so the output projection for a q-chunk can overlap the
            # next chunk's attention. Inner loop is pipelined per kt-triplet:
            # ring slots of 2 units per kt (pos 0-1 / 2-3 / 4-5); exp for kts
            # 0,1 of a triplet is one ScalarE ACTIVATE over ring[0:2048], kt 2
            # goes to VectorE via a Schraudolph fast-exp (int16 bit trick ->
            # bf16), offloading 1/3 of the exp work so ScalarE never gates the
            # PE. PV matmuls trail by one triplet.
            with ExitStack() as s23:
                ctxT_pool = s23.enter_context(tc.tile_pool(name="ctxT", bufs=3))
                ctxT = [ctxT_pool.tile([128, N], BF16, tag="ctxT",
                                       name=f"ctxT{i}") for i in range(3)]

                with ExitStack() as s2, ExitStack() as s3:
                    ctx_pool = s2.enter_context(
                        tc.tile_pool(name="ctxps", bufs=2, space="PSUM"))
                    exp1_pool = s2.enter_context(tc.tile_pool(name="et1", bufs=10))
                    small_pool = s2.enter_context(tc.tile_pool(name="small", bufs=4))
                    tmp_pool = s2.enter_context(tc.tile_pool(name="ctmp", bufs=2))
                    out_pool = s3.enter_context(tc.tile_pool(name="outT", bufs=4))

                    def _emit_proj(qcp):
                        # E: out^T = W_proj^T ctx^T (+bias) for q chunk qcp
                        for of in range(C // 128):
                            ps, _ = ring.unit()
                            for c2 in range(3):
                                nc.tensor.matmul(
                                    ps[:], wp[c2][:, of * 128:(of + 1) * 128],
                                    ctxT[c2][:, qcp * 512:(qcp + 1) * 512],
                                    start=(c2 == 0), stop=(c2 == 2))
                            ot = out_pool.tile([128, 512], F32R, tag="outT",
                                               name="ot")
                            nc.vector.tensor_scalar_add(ot[:], ps[:],
                                                        b_p[:, of:of + 1])
                            nc.sync.dma_start(
                                out_d[of * 128:(of + 1) * 128,
                                      qcp * 512:(qcp + 1) * 512], ot[:])

                    pv_defer = []   # deferred PV work: (kk, [et_ap0, et_ap1])

                    def _emit_pv(batch):
                        for ctxps, et_ap, kk, lh in batch["work"]:
                            nc.tensor.matmul(
                                ctxps[:],
                                vaug[kk][:, lh * 65:(lh + 1) * 65],
                                et_ap,
                                start=(kk == 0), stop=(kk == KT - 1))
                        if batch["evac"] is not None:
                            batch["evac"]()

                    def _make_evac(hp, qc, ctxps):
                        def _evac():
                            # normalize straight out of PSUM: 1/rowsum,
                            # broadcast, multiply. (reciprocal_approx_fast
                            # needs an SBUF source: copy the rowsum rows out.)
                            rsA = small_pool.tile([1, 512], F32, tag="rsA")
                            rsB = small_pool.tile([1, 512], F32, tag="rsB")
                            nc.vector.tensor_copy(rsA[:], ctxps[0][64:65, :])
                            nc.vector.tensor_copy(rsB[:], ctxps[1][64:65, :])
                            recipA = small_pool.tile([1, 512], F32, tag="recipA")
                            recipB = small_pool.tile([1, 512], F32, tag="recipB")
                            nc.vector.reciprocal_approx_fast(recipA[:], rsA[:])
                            nc.vector.reciprocal_approx_fast(recipB[:], rsB[:])
                            bcA = small_pool.tile([64, 512], F32, tag="bcA")
                            bcB = small_pool.tile([64, 512], F32, tag="bcB")
                            nc.gpsimd.partition_broadcast(bcA[:], recipA[:])
                            nc.gpsimd.partition_broadcast(bcB[:], recipB[:])
                            nc.vector.scalar_tensor_tensor(
                                ctxT[hp][0:64, qc * 512:(qc + 1) * 512],
                                ctxps[0][0:64, :], 1.0, bcA[:],
                                op0=ALU.mult, op1=ALU.mult)
                            ctmp = tmp_pool.tile([64, 512], BF16, tag="ctmp",
                                                 name="ctmp")
                            nc.vector.scalar_tensor_tensor(
                                ctmp[:], ctxps[1][0:64, :], 1.0, bcB[:],
                                op0=ALU.mult, op1=ALU.mult)
                            nc.sync.dma_start(
                                ctxT[hp][64:128, qc * 512:(qc + 1) * 512],
                                ctmp[:])
                            nc.sync.dma_start(
                                aot_d[hp * 128:(hp + 1) * 128,
                                      qc * 512:(qc + 1) * 512],
                                ctxT[hp][:, qc * 512:(qc + 1) * 512])
                        return _evac

                    for qc in range(QC):
                        for hp in range(3):
                            if hp == 1 and qc > 0:
                                _emit_proj(qc - 1)
                            ring.align(6)
                            ctxps = [ctx_pool.tile([65, 512], F32, tag="ctxps",
                                                   name=f"ctxps{i}")
                                     for i in range(2)]
                            for kt in range(KT):
                                r = kt % 3
                                # flush deferred PV before the ST that may
                                # wait on a ring slot, so the PE has work
                                # queued ahead of the wait
                                while len(pv_defer) > 4:
                                    _emit_pv(pv_defer.pop(0))
                                for ab in range(2):
                                    sts, pos = ring.unit()
                                    ho = ab * 64
                                    nc.tensor.matmul(
                                        sts,
                                        qkT[3 + hp][ho:ho + 64, kt * 128:(kt + 1) * 128],
                                        qkT[hp][ho:ho + 64, qc * 512:(qc + 1) * 512],
                                        start=True, stop=True,
                                        tile_position=(ho, 0))
                                assert pos == 2 * r + 1
                                et1 = exp1_pool.tile([128, 1024], BF16,
                                                     tag="et1", name="et1")
                                if r == 0:
                                    # DVE fast-exp takes every third kt
                                    # (and the last), offloading ScalarE
                                    nc.vector.tensor_scalar(
                                        et1[:].bitcast(I16),
                                        ring.t[:, r * 1024:(r + 1) * 1024],
                                        FE_A, FE_B,
                                        op0=ALU.mult, op1=ALU.add)
                                else:
                                    nc.scalar.activation(
                                        et1[:], ring.t[:, r * 1024:(r + 1) * 1024],
                                        AF.Exp)
                                batch = {
                                    "work": [
                                        (ctxps[0], et1[:, 0:512], kt, hp * 2),
                                        (ctxps[1], et1[:, 512:1024], kt,
                                         hp * 2 + 1)],
                                    "evac": None}
                                if kt == KT - 1:
                                    batch["evac"] = _make_evac(hp, qc, ctxps)
                                pv_defer.append(batch)
                            # drain: evac must reach the PE stream promptly
                            # so the accumulators free for the next round
                            while pv_defer:
                                _emit_pv(pv_defer.pop(0))
                    _emit_proj(QC - 1)


    nc.compile()
    return nc


def _get_nc(repeat=1):
    key = ("nc", repeat)
    if key not in _CACHE:
        _CACHE[key] = _build_nc(repeat)
    return _CACHE[key]


def _prep_inputs(x, W_qkv, b_qkv, W_proj, b_proj):
    x = np.ascontiguousarray(np.asarray(x, dtype=np.float32))
    W_qkv = np.asarray(W_qkv, dtype=np.float32)
    b_qkv = np.asarray(b_qkv, dtype=np.float32)
    W_proj = np.asarray(W_proj, dtype=np.float32)
    b_proj = np.asarray(b_proj, dtype=np.float32)

    bf = ml_dtypes.bfloat16
    in_maps = []
    for c in range(N_CORES):
        b, g = divmod(c, G)
        sl = slice(g * CL, (g + 1) * CL)
        w_q = W_qkv[:, 0:C][:, sl] * SCALE
        w_k = W_qkv[:, C:2 * C][:, sl]
        w_v = np.ascontiguousarray(W_qkv[:, 2 * C:3 * C][:, sl])
        b_q = b_qkv[0:C][sl] * SCALE
        b_k = b_qkv[C:2 * C][sl]
        b_v = b_qkv[2 * C:3 * C][sl]
        w_qk = np.ascontiguousarray(np.concatenate([w_q, w_k], axis=1))
        b_qk = np.ascontiguousarray(
            np.concatenate([b_q, b_k]).reshape(FQK, 128).T)
        w_p = np.ascontiguousarray(W_proj[sl, :])
        bp = b_proj if g == 0 else np.zeros_like(b_proj)
        b_p = np.ascontiguousarray(bp.reshape(C // 128, 128).T)
        in_maps.append({
            "xT": np.ascontiguousarray(x[b].T).astype(bf),
            "w_qk": w_qk.astype(bf),
            "w_v": w_v.astype(bf),
            "w_p": w_p.astype(bf),
            "b_qk": b_qk,
            "b_v": np.ascontiguousarray(b_v[None, :]).astype(bf),
            "b_p": b_p,
            "ones_col": np.ones((1, 128), dtype=bf),
        })
    return in_maps


def run_cores(in_maps, **kw):
    nc = _get_nc()
    return run_bass_kernel_spmd(nc, in_maps, list(range(N_CORES)), **kw)


def gather(results):
    out = np.empty((B, N, C), dtype=np.float32)
    attn_out = np.empty((B, N, C), dtype=np.float32)
    for b in range(B):
        r0 = results[b * G + 0]
        r1 = results[b * G + 1]
        attn_out[b, :, 0:CL] = r0["attn_out_t"].T
        attn_out[b, :, CL:C] = r1["attn_out_t"].T
        out[b] = r0["out_t"].T
        out[b] += r1["out_t"].T
    return out, attn_out


def kernel(x, W_qkv, b_qkv, W_proj, b_proj):
    in_maps = _prep_inputs(x, W_qkv, b_qkv, W_proj, b_proj)
    res = run_cores(in_maps)
    return gather(res.results)



# revision 23
# speedup vs baseline: 1.7319x; 1.7319x over previous
"""Fused multi-head attention + output projection for Trainium2 (Bass/Tile).

Problem: B=4, N=2048, C=768, H=12 heads x D=64.
  qkv = x @ W_qkv + b_qkv ; q,k,v per head ; attn = softmax(q k^T / sqrt(D))
  attn_out = (attn @ v) merged ; out = attn_out @ W_proj + b_proj
  returns (out, attn_out)

Sharding over 8 NeuronCores: core c = (b, g) with b = batch (4), g = head
group (2 groups of 6 heads).  Data-parallel over batch, tensor-parallel over
heads: W_qkv columns / W_proj rows are split per group; the N x N attention
matrix stays core-local.  Host only slices inputs and, on gather, transposes
the (feature-major) outputs and sums the two W_proj partial products per
batch.

Per-core device algorithm (all layouts feature-major "T" = [features, n]):
  xT = transpose(x_b)                       (PE transposes via identity)
  qkT[f, n] = W_qk^T x (+bias, q pre-scaled on host)      fp32r matmuls
  v[n, f] (+bias via ones-row matmul), stored bf16 augmented with a ones
      column per head -> PV matmul also yields softmax row-sums.
  Per head: S^T[k, q] = kT^T qT (no max subtraction needed: |S| <= ~6),
      P^T = exp(S^T) on ScalarE straight out of PSUM (bf16),
      ctx^T[d, q] (+rowsum row) = [v|1]^T @ P^T, normalize by 1/rowsum.
  out^T = W_proj^T ctx^T (+b_proj on group-0 cores only, via zeroed input).

All phases share one 6-bank PSUM "ring" of [128,512] units (plus 2 banks of
PV accumulators), so no PSUM pool boundary serializes phase transitions.
"""

import os
import numpy as np
import ml_dtypes
from contextlib import ExitStack

import concourse.bass as bass
import concourse.tile as tile
import concourse.mybir as mybir
from concourse import bacc
import concourse.bass_utils as _bass_utils
from concourse.bass_utils import run_bass_kernel_spmd

# walrus is invoked with --enable-ldw-opt=false by default, which forces a
# serial LDWEIGHTS before every MATMUL (~250us of PE time for this kernel).
_orig_run_command = _bass_utils.run_command


def _run_command_ldw(argv, **kw):
    argv = ["--enable-ldw-opt=true" if a == "--enable-ldw-opt=false" else a
            for a in argv]
    return _orig_run_command(argv, **kw)


# NOTE: tried --enable-ldw-opt=true: walrus rejects it for fp32/fp32r
# weights ("InstLdweights is not compatible with LDW optimization").
ENABLE_LDW_OPT = bool(os.environ.get("K_LDW_OPT"))
if ENABLE_LDW_OPT and _bass_utils.run_command is _orig_run_command:
    _bass_utils.run_command = _run_command_ldw

N_CORES = 8
B, N, C = 4, 2048, 768
H, D = 12, 64
G = 2                # head groups (tensor-parallel)
HL = H // G          # heads per core
CL = HL * D          # local feature width (384)
SCALE = D ** -0.5
NT = N // 128        # 16 row tiles
CC = C // 128        # 6 contraction chunks
QC = N // 512        # 4 q chunks of 512
KT = N // 128        # 16 k tiles
FQK = 2 * CL // 128  # 6 feature tiles for q|k

F32 = mybir.dt.float32
F32R = mybir.dt.float32r
BF16 = mybir.dt.bfloat16
I16 = mybir.dt.int16

# Schraudolph fast-exp in bf16 bit space: bf16_bits(exp(x)) ~ x*FE_A + FE_B.
# FE_B calibrated on HW (trunc semantics) for zero-mean relative error.
FE_A = 2.0 ** 7 / float(np.log(2.0))
FE_B = 16256.5 - 7.88
AF = mybir.ActivationFunctionType
ALU = mybir.AluOpType

_CACHE = {}


class Ring:
    """Rotating [128, 512] PSUM units across three persistent 2-bank
    slot tiles. Separate tiles keep the tile-level dependency tracking
    per-slot: the ST matmul reusing slot s waits only on that slot's
    exp reader three kts back, not on every in-flight ring access."""

    def __init__(self, slots):
        self.slots = slots          # 3 tiles of [128, 1024]
        self.pos = 0

    def unit(self, width=512):
        p = self.pos % 6
        self.pos += 1
        return self.slots[p // 2][:, (p % 2) * 512:(p % 2) * 512 + width], p

    def slot(self, r):
        return self.slots[r]

    def align(self, m):
        self.pos = ((self.pos + m - 1) // m) * m


def _build_nc(repeat=1):
    nc = bacc.Bacc("TRN2", target_bir_lowering=False, debug=False,
                   num_devices=N_CORES)
    xt_d = nc.dram_tensor("xT", [C, N], BF16, kind="ExternalInput").ap()
    wqk_d = nc.dram_tensor("w_qk", [C, 2 * CL], BF16, kind="ExternalInput").ap()
    wv_d = nc.dram_tensor("w_v", [C, CL], BF16, kind="ExternalInput").ap()
    wp_d = nc.dram_tensor("w_p", [CL, C], BF16, kind="ExternalInput").ap()
    bqk_d = nc.dram_tensor("b_qk", [128, FQK], F32, kind="ExternalInput").ap()
    bv_d = nc.dram_tensor("b_v", [1, CL], BF16, kind="ExternalInput").ap()
    bp_d = nc.dram_tensor("b_p", [128, C // 128], F32, kind="ExternalInput").ap()
    onesc_d = nc.dram_tensor("ones_col", [1, 128], BF16, kind="ExternalInput").ap()
    aot_d = nc.dram_tensor("attn_out_t", [CL, N], BF16, kind="ExternalOutput").ap()
    out_d = nc.dram_tensor("out_t", [C, N], F32R, kind="ExternalOutput").ap()
    DEBUG = bool(os.environ.get("K_DEBUG"))
    if DEBUG:
        dbg_rs = nc.dram_tensor("dbg_rs", [1, 512], F32, kind="ExternalOutput").ap()
        dbg_rA = nc.dram_tensor("dbg_rA", [1, 512], F32, kind="ExternalOutput").ap()
        dbg_bc = nc.dram_tensor("dbg_bc", [64, 512], F32, kind="ExternalOutput").ap()

    with tile.TileContext(nc) as tc:
      for _rep in range(repeat):
        with ExitStack() as top:
            const_pool = top.enter_context(tc.tile_pool(name="const", bufs=1))
            bias_pool = top.enter_context(tc.tile_pool(name="bias", bufs=3))
            qkT_pool = top.enter_context(tc.tile_pool(name="qkT", bufs=FQK))
            vaug_pool = top.enter_context(tc.tile_pool(name="vaug", bufs=NT))
            wp_pool = top.enter_context(tc.tile_pool(name="wp", bufs=3))
            ring_pool = top.enter_context(
                tc.tile_pool(name="ring", bufs=1, space="PSUM"))

            ring = Ring([ring_pool.tile([128, 1024], F32, tag=f"ring{i}",
                                        name=f"ringt{i}") for i in range(3)])

            # x arrives pre-transposed from the host: plain parallel DMA
            # loads instead of the serialized transpose-xbar path.
            xT_pool_o = top.enter_context(tc.tile_pool(name="xT", bufs=CC))
            xT = [xT_pool_o.tile([128, N], BF16, tag="xT", name=f"xTt{i}")
                  for i in range(CC)]
            for cc in range(CC):
                nc.sync.dma_start(xT[cc][:], xt_d[cc * 128:(cc + 1) * 128, :])

            ones_col = const_pool.tile([1, 128], BF16, tag="ones")
            nc.sync.dma_start(ones_col[:], onesc_d[:])

            b_qk = bias_pool.tile([128, FQK], F32, tag="bqk")
            nc.sync.dma_start(b_qk[:], bqk_d[:])
            b_p = bias_pool.tile([128, C // 128], F32, tag="bp")
            nc.sync.dma_start(b_p[:], bp_d[:])
            b_v = bias_pool.tile([1, CL], BF16, tag="bv")
            nc.sync.dma_start(b_v[:], bv_d[:])

            wp = []
            for i in range(3):
                t = wp_pool.tile([128, C], BF16, tag="wp", name=f"wp{i}")
                nc.sync.dma_start(t[:], wp_d[i * 128:(i + 1) * 128, :])
                wp.append(t)

            qkT = [qkT_pool.tile([128, N], BF16, tag="qkT", name=f"qkT{i}")
                   for i in range(FQK)]
            vaug = [vaug_pool.tile([128, HL * 65], BF16, tag="vaug",
                                   name=f"vaug{i}") for i in range(NT)]
            # ------------- Phases A (xT), B (qkT), C (v) -------------
            with ExitStack() as s1:
                wv_pool = s1.enter_context(tc.tile_pool(name="wv", bufs=CC))

                wv = []
                for cc in range(CC):
                    t = wv_pool.tile([128, CL], BF16, tag="wv", name=f"wv{cc}")
                    nc.sync.dma_start(t[:], wv_d[cc * 128:(cc + 1) * 128, :])
                    wv.append(t)

                with ExitStack() as s1a:
                    wqk_pool = s1a.enter_context(
                        tc.tile_pool(name="wqk", bufs=CC))
                    wqk = []
                    for cc in range(CC):
                        t = wqk_pool.tile([128, 2 * CL], BF16, tag="wqk",
                                          name=f"wqk{cc}")
                        nc.sync.dma_start(t[:], wqk_d[cc * 128:(cc + 1) * 128, :])
                        wqk.append(t)


                    # B: qkT = W_qk^T @ x^T (+ per-partition bias on evac)
                    for ft in range(FQK):
                        for qc in range(QC):
                            ps, _ = ring.unit()
                            for cc in range(CC):
                                nc.tensor.matmul(
                                    ps[:], wqk[cc][:, ft * 128:(ft + 1) * 128],
                                    xT[cc][:, qc * 512:(qc + 1) * 512],
                                    start=(cc == 0), stop=(cc == CC - 1))
                            if qc % 2 == 0:
                                nc.vector.tensor_scalar_add(
                                    qkT[ft][:, qc * 512:(qc + 1) * 512], ps[:],
                                    b_qk[:, ft:ft + 1])
                            else:
                                nc.scalar.activation(
                                    qkT[ft][:, qc * 512:(qc + 1) * 512], ps[:],
                                    AF.Identity, bias=b_qk[:, ft:ft + 1])

                # C: v natural (+bias via ones-row), ones col per head
                for nt in range(NT):
                    ps, _ = ring.unit(width=CL)
                    for cc in range(CC):
                        nc.tensor.matmul(
                            ps[:], xT[cc][:, nt * 128:(nt + 1) * 128], wv[cc][:],
                            start=(cc == 0), stop=False)
                    nc.tensor.matmul(ps[:], ones_col[:], b_v[:],
                                     start=False, stop=True)
                    va3 = vaug[nt][:].rearrange("p (h e) -> p h e", e=65)
                    if nt % 2 == 0:
                        nc.vector.tensor_copy(
                            va3[:, :, 0:64],
                            ps[:].rearrange("p (h e) -> p h e", e=64))
                    else:
                        nc.scalar.activation(
                            va3[:, :, 0:64],
                            ps[:].rearrange("p (h e) -> p h e", e=64),
                            AF.Identity)
                    nc.vector.memset(va3[:, :, 64:65], 1.0)

            # ---------------- Phases D (attention) + E (proj) ----------------
            # qc-major so the output projection for a q-chunk can overlap the
            # next chunk's attention. Inner loop is pipelined per kt-triplet:
            # ring slots of 2 units per kt (pos 0-1 / 2-3 / 4-5); exp for kts
            # 0,1 of a triplet is one ScalarE ACTIVATE over ring[0:2048], kt 2
            # goes to VectorE via a Schraudolph fast-exp (int16 bit trick ->
            # bf16), offloading 1/3 of the exp work so ScalarE never gates the
            # PE. PV matmuls trail by one triplet.
            with ExitStack() as s23:
                ctxT_pool = s23.enter_context(tc.tile_pool(name="ctxT", bufs=3))
                ctxT = [ctxT_pool.tile([128, N], BF16, tag="ctxT",
                                       name=f"ctxT{i}") for i in range(3)]

                with ExitStack() as s2, ExitStack() as s3:
                    ctx_pool = s2.enter_context(
                        tc.tile_pool(name="ctxps", bufs=2, space="PSUM"))
                    exp1_pool = s2.enter_context(tc.tile_pool(name="et1", bufs=10))
                    small_pool = s2.enter_context(tc.tile_pool(name="small", bufs=4))
                    tmp_pool = s2.enter_context(tc.tile_pool(name="ctmp", bufs=2))
                    out_pool = s3.enter_context(tc.tile_pool(name="outT", bufs=4))

                    def _emit_proj(qcp):
                        # E: out^T = W_proj^T ctx^T (+bias) for q chunk qcp
                        for of in range(C // 128):
                            ps, _ = ring.unit()
                            for c2 in range(3):
                                nc.tensor.matmul(
                                    ps[:], wp[c2][:, of * 128:(of + 1) * 128],
                                    ctxT[c2][:, qcp * 512:(qcp + 1) * 512],
                                    start=(c2 == 0), stop=(c2 == 2))
                            ot = out_pool.tile([128, 512], F32R, tag="outT",
                                               name="ot")
                            nc.vector.tensor_scalar_add(ot[:], ps[:],
                                                        b_p[:, of:of + 1])
                            nc.sync.dma_start(
                                out_d[of * 128:(of + 1) * 128,
                                      qcp * 512:(qcp + 1) * 512], ot[:])

                    pv_defer = []   # deferred PV work: (kk, [et_ap0, et_ap1])

                    def _emit_pv(batch):
                        for ctxps, et_ap, kk, lh in batch["work"]:
                            nc.tensor.matmul(
                                ctxps[:],
                                vaug[kk][:, lh * 65:(lh + 1) * 65],
                                et_ap,
                                start=(kk == 0), stop=(kk == KT - 1))
                        if batch["evac"] is not None:
                            batch["evac"]()

                    def _make_evac(hp, qc, ctxps):
                        def _evac():
                            # normalize straight out of PSUM: 1/rowsum,
                            # broadcast, multiply. (reciprocal_approx_fast
                            # needs an SBUF source: copy the rowsum rows out.)
                            rsA = small_pool.tile([1, 512], F32, tag="rsA")
                            rsB = small_pool.tile([1, 512], F32, tag="rsB")
                            nc.vector.tensor_copy(rsA[:], ctxps[0][64:65, :])
                            nc.vector.tensor_copy(rsB[:], ctxps[1][64:65, :])
                            recipA = small_pool.tile([1, 512], F32, tag="recipA")
                            recipB = small_pool.tile([1, 512], F32, tag="recipB")
                            nc.vector.reciprocal_approx_fast(recipA[:], rsA[:])
                            nc.vector.reciprocal_approx_fast(recipB[:], rsB[:])
                            bcA = small_pool.tile([64, 512], F32, tag="bcA")
                            bcB = small_pool.tile([64, 512], F32, tag="bcB")
                            nc.gpsimd.partition_broadcast(bcA[:], recipA[:])
                            nc.gpsimd.partition_broadcast(bcB[:], recipB[:])
                            nc.vector.scalar_tensor_tensor(
                                ctxT[hp][0:64, qc * 512:(qc + 1) * 512],
                                ctxps[0][0:64, :], 1.0, bcA[:],
                                op0=ALU.mult, op1=ALU.mult)
                            ctmp = tmp_pool.tile([64, 512], BF16, tag="ctmp",
                                                 name="ctmp")
                            nc.vector.scalar_tensor_tensor(
                                ctmp[:], ctxps[1][0:64, :], 1.0, bcB[:],
                                op0=ALU.mult, op1=ALU.mult)
                            nc.sync.dma_start(
                                ctxT[hp][64:128, qc * 512:(qc + 1) * 512],
                                ctmp[:])
                            nc.sync.dma_start(
                                aot_d[hp * 128:(hp + 1) * 128,
                                      qc * 512:(qc + 1) * 512],
                                ctxT[hp][:, qc * 512:(qc + 1) * 512])
                        return _evac

                    for qc in range(QC):
                        for hp in range(3):
                            if hp == 1 and qc > 0:
                                _emit_proj(qc - 1)
                            ring.align(6)
                            ctxps = [ctx_pool.tile([65, 512], F32, tag="ctxps",
                                                   name=f"ctxps{i}")
                                     for i in range(2)]
                            for kt in range(KT):
                                r = kt % 3
                                # flush deferred PV before the ST that may
                                # wait on a ring slot, so the PE has work
                                # queued ahead of the wait
                                while len(pv_defer) > 4:
                                    _emit_pv(pv_defer.pop(0))
                                for ab in range(2):
                                    sts, pos = ring.unit()
                                    ho = ab * 64
                                    nc.tensor.matmul(
                                        sts,
                                        qkT[3 + hp][ho:ho + 64, kt * 128:(kt + 1) * 128],
                                        qkT[hp][ho:ho + 64, qc * 512:(qc + 1) * 512],
                                        start=True, stop=True,
                                        tile_position=(ho, 0))
                                assert pos == 2 * r + 1
                                et1 = exp1_pool.tile([128, 1024], BF16,
                                                     tag="et1", name="et1")
                                if r == 0:
                                    # DVE fast-exp takes every third kt
                                    # (and the last), offloading ScalarE
                                    nc.vector.tensor_scalar(
                                        et1[:].bitcast(I16),
                                        ring.slot(r)[:],
                                        FE_A, FE_B,
                                        op0=ALU.mult, op1=ALU.add)
                                else:
                                    nc.scalar.activation(
                                        et1[:], ring.slot(r)[:], AF.Exp)
                                batch = {
                                    "work": [
                                        (ctxps[0], et1[:, 0:512], kt, hp * 2),
                                        (ctxps[1], et1[:, 512:1024], kt,
                                         hp * 2 + 1)],
                                    "evac": None}
                                if kt == KT - 1:
                                    batch["evac"] = _make_evac(hp, qc, ctxps)
                                pv_defer.append(batch)
                            # drain: evac must reach the PE stream promptly
                            # so the accumulators free for the next round
                            while pv_defer:
                                _emit_pv(pv_defer.pop(0))
                    _emit_proj(QC - 1)


    nc.compile()
    return nc


def _get_nc(repeat=1):
    key = ("nc", repeat)
    if key not in _CACHE:
        _CACHE[key] = _build_nc(repeat)
    return _CACHE[key]


def _prep_inputs(x, W_qkv, b_qkv, W_proj, b_proj):
    x = np.ascontiguousarray(np.asarray(x, dtype=np.float32))
    W_qkv = np.asarray(W_qkv, dtype=np.float32)
    b_qkv = np.asarray(b_qkv, dtype=np.float32)
    W_proj = np.asarray(W_proj, dtype=np.float32)
    b_proj = np.asarray(b_proj, dtype=np.float32)

    bf = ml_dtypes.bfloat16
    in_maps = []
    for c in range(N_CORES):
        b, g = divmod(c, G)
        sl = slice(g * CL, (g + 1) * CL)
        w_q = W_qkv[:, 0:C][:, sl] * SCALE
        w_k = W_qkv[:, C:2 * C][:, sl]
        w_v = np.ascontiguousarray(W_qkv[:, 2 * C:3 * C][:, sl])
        b_q = b_qkv[0:C][sl] * SCALE
        b_k = b_qkv[C:2 * C][sl]
        b_v = b_qkv[2 * C:3 * C][sl]
        w_qk = np.ascontiguousarray(np.concatenate([w_q, w_k], axis=1))
        b_qk = np.ascontiguousarray(
            np.concatenate([b_q, b_k]).reshape(FQK, 128).T)
        w_p = np.ascontiguousarray(W_proj[sl, :])
        bp = b_proj if g == 0 else np.zeros_like(b_proj)
        b_p = np.ascontiguousarray(bp.reshape(C // 128, 128).T)
        in_maps.append({
            "xT": np.ascontiguousarray(x[b].T).astype(bf),
            "w_qk": w_qk.astype(bf),
            "w_v": w_v.astype(bf),
            "w_p": w_p.astype(bf),
            "b_qk": b_qk,
            "b_v": np.ascontiguousarray(b_v[None, :]).astype(bf),
            "b_p": b_p,
            "ones_col": np.ones((1, 128), dtype=bf),
        })
    return in_maps


def run_cores(in_maps, **kw):
    nc = _get_nc()
    return run_bass_kernel_spmd(nc, in_maps, list(range(N_CORES)), **kw)


def gather(results):
    out = np.empty((B, N, C), dtype=np.float32)
    attn_out = np.empty((B, N, C), dtype=np.float32)
    for b in range(B):
        r0 = results[b * G + 0]
        r1 = results[b * G + 1]
        attn_out[b, :, 0:CL] = r0["attn_out_t"].T
        attn_out[b, :, CL:C] = r1["attn_out_t"].T
        out[b] = r0["out_t"].T
        out[b] += r1["out_t"].T
    return out, attn_out


def kernel(x, W_qkv, b_qkv, W_proj, b_proj):
    in_maps = _prep_inputs(x, W_qkv, b_qkv, W_proj, b_proj)
    res = run_cores(in_maps)
    return gather(res.results)



# revision 32
# speedup vs baseline: 1.7862x; 1.0314x over previous
"""Fused multi-head attention + output projection for Trainium2 (Bass/Tile).

Problem: B=4, N=2048, C=768, H=12 heads x D=64.
  qkv = x @ W_qkv + b_qkv ; q,k,v per head ; attn = softmax(q k^T / sqrt(D))
  attn_out = (attn @ v) merged ; out = attn_out @ W_proj + b_proj
  returns (out, attn_out)

Sharding over 8 NeuronCores: core c = (b, g) with b = batch (4), g = head
group (2 groups of 6 heads).  Data-parallel over batch, tensor-parallel over
heads: W_qkv columns / W_proj rows are split per group; the N x N attention
matrix stays core-local.  Host only slices inputs and, on gather, transposes
the (feature-major) outputs and sums the two W_proj partial products per
batch.

Per-core device algorithm (all layouts feature-major "T" = [features, n]):
  xT = transpose(x_b)                       (PE transposes via identity)
  qkT[f, n] = W_qk^T x (+bias, q pre-scaled on host)      fp32r matmuls
  v[n, f] (+bias via ones-row matmul), stored bf16 augmented with a ones
      column per head -> PV matmul also yields softmax row-sums.
  Per head: S^T[k, q] = kT^T qT (no max subtraction needed: |S| <= ~6),
      P^T = exp(S^T) on ScalarE straight out of PSUM (bf16),
      ctx^T[d, q] (+rowsum row) = [v|1]^T @ P^T, normalize by 1/rowsum.
  out^T = W_proj^T ctx^T (+b_proj on group-0 cores only, via zeroed input).

All phases share one 6-bank PSUM "ring" of [128,512] units (plus 2 banks of
PV accumulators), so no PSUM pool boundary serializes phase transitions.
"""

import os
import numpy as np
import ml_dtypes
from contextlib import ExitStack

import concourse.bass as bass
import concourse.tile as tile
import concourse.mybir as mybir
from concourse import bacc
import concourse.bass_utils as _bass_utils
from concourse.bass_utils import run_bass_kernel_spmd

# walrus is invoked with --enable-ldw-opt=false by default, which forces a
# serial LDWEIGHTS before every MATMUL (~250us of PE time for this kernel).
_orig_run_command = _bass_utils.run_command


def _run_command_ldw(argv, **kw):
    argv = ["--enable-ldw-opt=true" if a == "--enable-ldw-opt=false" else a
            for a in argv]
    return _orig_run_command(argv, **kw)


# NOTE: tried --enable-ldw-opt=true: walrus rejects it for fp32/fp32r
# weights ("InstLdweights is not compatible with LDW optimization").
ENABLE_LDW_OPT = bool(os.environ.get("K_LDW_OPT"))
if ENABLE_LDW_OPT and _bass_utils.run_command is _orig_run_command:
    _bass_utils.run_command = _run_command_ldw

N_CORES = 8
B, N, C = 4, 2048, 768
H, D = 12, 64
G = 2                # head groups (tensor-parallel)
HL = H // G          # heads per core
CL = HL * D          # local feature width (384)
SCALE = D ** -0.5
NT = N // 128        # 16 row tiles
CC = C // 128        # 6 contraction chunks
QC = N // 512        # 4 q chunks of 512
KT = N // 128        # 16 k tiles
FQK = 2 * CL // 128  # 6 feature tiles for q|k

F32 = mybir.dt.float32
F32R = mybir.dt.float32r
BF16 = mybir.dt.bfloat16
I16 = mybir.dt.int16

# Schraudolph fast-exp in bf16 bit space: bf16_bits(exp(x)) ~ x*FE_A + FE_B.
# FE_B calibrated on HW (trunc semantics) for zero-mean relative error.
FE_A = 2.0 ** 7 / float(np.log(2.0))
FE_B = 16256.5 - 7.88
AF = mybir.ActivationFunctionType
ALU = mybir.AluOpType

_CACHE = {}


class Ring:
    """Rotating [128, 512] PSUM units across three persistent 2-bank
    slot tiles. Separate tiles keep the tile-level dependency tracking
    per-slot: the ST matmul reusing slot s waits only on that slot's
    exp reader three kts back, not on every in-flight ring access."""

    def __init__(self, slots):
        self.slots = slots          # 3 tiles of [128, 1024]
        self.pos = 0

    def unit(self, width=512):
        p = self.pos % 6
        self.pos += 1
        return self.slots[p // 2][:, (p % 2) * 512:(p % 2) * 512 + width], p

    def slot(self, r):
        return self.slots[r]

    def align(self, m):
        self.pos = ((self.pos + m - 1) // m) * m


def _build_nc(repeat=1):
    nc = bacc.Bacc("TRN2", target_bir_lowering=False, debug=False,
                   num_devices=N_CORES)
    xt_d = nc.dram_tensor("xT", [C, N], BF16, kind="ExternalInput").ap()
    wqk_d = nc.dram_tensor("w_qk", [C, 2 * CL], BF16, kind="ExternalInput").ap()
    wv_d = nc.dram_tensor("w_v", [C, CL], BF16, kind="ExternalInput").ap()
    wp_d = nc.dram_tensor("w_p", [CL, C], BF16, kind="ExternalInput").ap()
    bqk_d = nc.dram_tensor("b_qk", [128, FQK], F32, kind="ExternalInput").ap()
    bv_d = nc.dram_tensor("b_v", [1, CL], BF16, kind="ExternalInput").ap()
    bp_d = nc.dram_tensor("b_p", [128, C // 128], F32, kind="ExternalInput").ap()
    onesc_d = nc.dram_tensor("ones_col", [1, 128], BF16, kind="ExternalInput").ap()
    aot_d = nc.dram_tensor("attn_out_t", [CL, N], BF16, kind="ExternalOutput").ap()
    out_d = nc.dram_tensor("out_t", [C, N], F32R, kind="ExternalOutput").ap()
    DEBUG = bool(os.environ.get("K_DEBUG"))
    if DEBUG:
        dbg_rs = nc.dram_tensor("dbg_rs", [1, 512], F32, kind="ExternalOutput").ap()
        dbg_rA = nc.dram_tensor("dbg_rA", [1, 512], F32, kind="ExternalOutput").ap()
        dbg_bc = nc.dram_tensor("dbg_bc", [64, 512], F32, kind="ExternalOutput").ap()

    with tile.TileContext(nc) as tc:
      for _rep in range(repeat):
        with ExitStack() as top:
            const_pool = top.enter_context(tc.tile_pool(name="const", bufs=1))
            bias_pool = top.enter_context(tc.tile_pool(name="bias", bufs=3))
            qkT_pool = top.enter_context(tc.tile_pool(name="qkT", bufs=FQK))
            vaug_pool = top.enter_context(tc.tile_pool(name="vaug", bufs=NT))
            wp_pool = top.enter_context(tc.tile_pool(name="wp", bufs=3))
            ring_pool = top.enter_context(
                tc.tile_pool(name="ring", bufs=1, space="PSUM"))

            ring = Ring([ring_pool.tile([128, 1024], F32, tag=f"ring{i}",
                                        name=f"ringt{i}") for i in range(3)])

            # x arrives pre-transposed from the host: plain parallel DMA
            # loads instead of the serialized transpose-xbar path.
            xT_pool_o = top.enter_context(tc.tile_pool(name="xT", bufs=CC))
            xT = [xT_pool_o.tile([128, N], BF16, tag="xT", name=f"xTt{i}")
                  for i in range(CC)]

            ones_col = const_pool.tile([1, 128], BF16, tag="ones")
            b_qk = bias_pool.tile([128, FQK], F32, tag="bqk")
            b_p = bias_pool.tile([128, C // 128], F32, tag="bp")
            b_v = bias_pool.tile([1, CL], BF16, tag="bv")

            qkT = [qkT_pool.tile([128, N], BF16, tag="qkT", name=f"qkT{i}")
                   for i in range(FQK)]
            vaug = [vaug_pool.tile([128, HL * 128], BF16, tag="vaug",
                                   name=f"vaug{i}") for i in range(NT)]
            # ------------- Phases A (xT), B (qkT), C (v) -------------
            with ExitStack() as s1:
                wv_pool = s1.enter_context(tc.tile_pool(name="wv", bufs=CC))
                wv = [wv_pool.tile([128, CL], BF16, tag="wv", name=f"wv{cc}")
                      for cc in range(CC)]

                with ExitStack() as s1a:
                    wqk_pool = s1a.enter_context(
                        tc.tile_pool(name="wqk", bufs=CC))
                    wqk = [wqk_pool.tile([128, 2 * CL], BF16, tag="wqk",
                                         name=f"wqk{cc}") for cc in range(CC)]
                    # DMA issue order = need order: the B-phase matmuls
                    # consume (wqk[cc], xT[cc]) pairs first.
                    for cc in range(CC):
                        nc.sync.dma_start(wqk[cc][:],
                                          wqk_d[cc * 128:(cc + 1) * 128, :])
                        nc.sync.dma_start(xT[cc][:],
                                          xt_d[cc * 128:(cc + 1) * 128, :])
                    nc.sync.dma_start(b_qk[:], bqk_d[:])
                    for cc in range(CC):
                        nc.sync.dma_start(wv[cc][:],
                                          wv_d[cc * 128:(cc + 1) * 128, :])
                    nc.sync.dma_start(ones_col[:], onesc_d[:])
                    nc.sync.dma_start(b_v[:], bv_d[:])
                    nc.sync.dma_start(b_p[:], bp_d[:])
                    wp = []
                    for i in range(3):
                        t = wp_pool.tile([128, C], BF16, tag="wp", name=f"wp{i}")
                        nc.sync.dma_start(t[:], wp_d[i * 128:(i + 1) * 128, :])
                        wp.append(t)

                    # B: qkT = W_qk^T @ x^T (+ per-partition bias on evac)
                    for ft in range(FQK):
                        for qc in range(QC):
                            ps, _ = ring.unit()
                            for cc in range(CC):
                                nc.tensor.matmul(
                                    ps[:], wqk[cc][:, ft * 128:(ft + 1) * 128],
                                    xT[cc][:, qc * 512:(qc + 1) * 512],
                                    start=(cc == 0), stop=(cc == CC - 1))
                            if qc % 2 == 0:
                                nc.vector.tensor_scalar_add(
                                    qkT[ft][:, qc * 512:(qc + 1) * 512], ps[:],
                                    b_qk[:, ft:ft + 1])
                            else:
                                nc.scalar.activation(
                                    qkT[ft][:, qc * 512:(qc + 1) * 512], ps[:],
                                    AF.Identity, bias=b_qk[:, ft:ft + 1])

                # C: v natural (+bias via ones-row), ones col per head
                for nt in range(NT):
                    ps, _ = ring.unit(width=CL)
                    for cc in range(CC):
                        nc.tensor.matmul(
                            ps[:], xT[cc][:, nt * 128:(nt + 1) * 128], wv[cc][:],
                            start=(cc == 0), stop=False)
                    nc.tensor.matmul(ps[:], ones_col[:], b_v[:],
                                     start=False, stop=True)
                    # vaug is padded to 128 cols per head (FWL-eligible
                    # ldweights): [v(64) | ones(1) | zeros(63)]
                    va3 = vaug[nt][:].rearrange("p (h e) -> p h e", e=128)
                    if nt % 2 == 0:
                        nc.vector.tensor_copy(
                            va3[:, :, 0:64],
                            ps[:].rearrange("p (h e) -> p h e", e=64))
                    else:
                        nc.scalar.activation(
                            va3[:, :, 0:64],
                            ps[:].rearrange("p (h e) -> p h e", e=64),
                            AF.Identity)
                    nc.vector.memset(va3[:, :, 64:65], 1.0)
                    nc.gpsimd.memset(va3[:, :, 65:128], 0.0)

            # ---------------- Phases D (attention) + E (proj) ----------------
            # qc-major so the output projection for a q-chunk can overlap the
            # next chunk's attention. Inner loop is pipelined per kt-triplet:
            # ring slots of 2 units per kt (pos 0-1 / 2-3 / 4-5); exp for kts
            # 0,1 of a triplet is one ScalarE ACTIVATE over ring[0:2048], kt 2
            # goes to VectorE via a Schraudolph fast-exp (int16 bit trick ->
            # bf16), offloading 1/3 of the exp work so ScalarE never gates the
            # PE. PV matmuls trail by one triplet.
            with ExitStack() as s23:
                ctxT_pool = s23.enter_context(tc.tile_pool(name="ctxT", bufs=3))
                ctxT = [ctxT_pool.tile([128, N], BF16, tag="ctxT",
                                       name=f"ctxT{i}") for i in range(3)]

                with ExitStack() as s2, ExitStack() as s3:
                    ctx_pool = s2.enter_context(
                        tc.tile_pool(name="ctxps", bufs=2, space="PSUM"))
                    exp1_pool = s2.enter_context(tc.tile_pool(name="et1", bufs=10))
                    small_pool = s2.enter_context(tc.tile_pool(name="small", bufs=4))
                    tmp_pool = s2.enter_context(tc.tile_pool(name="ctmp", bufs=2))
                    out_pool = s3.enter_context(tc.tile_pool(name="outT", bufs=4))

                    def _emit_proj(qcp):
                        # E: out^T = W_proj^T ctx^T (+bias) for q chunk qcp
                        for of in range(C // 128):
                            ps, _ = ring.unit()
                            for c2 in range(3):
                                nc.tensor.matmul(
                                    ps[:], wp[c2][:, of * 128:(of + 1) * 128],
                                    ctxT[c2][:, qcp * 512:(qcp + 1) * 512],
                                    start=(c2 == 0), stop=(c2 == 2))
                            ot = out_pool.tile([128, 512], F32R, tag="outT",
                                               name="ot")
                            nc.vector.tensor_scalar_add(ot[:], ps[:],
                                                        b_p[:, of:of + 1])
                            nc.sync.dma_start(
                                out_d[of * 128:(of + 1) * 128,
                                      qcp * 512:(qcp + 1) * 512], ot[:])

                    pv_defer = []   # deferred PV work: (kk, [et_ap0, et_ap1])
                    pending_evac = []  # evacs held until after the next
                                       # iteration's first DVE exp (keeps the
                                       # evac chain out of the fast-exp's way
                                       # in the Vector FIFO)

                    def _emit_pv(batch):
                        for ctxps, et_ap, kk, lh in batch["work"]:
                            nc.tensor.matmul(
                                ctxps[:],
                                vaug[kk][:, lh * 128:(lh + 1) * 128],
                                et_ap,
                                start=(kk == 0), stop=(kk == KT - 1))
                        if batch["evac"] is not None:
                            pending_evac.append(batch["evac"])

                    def _make_evac(hp, qc, ctxps):
                        def _evac():
                            # normalize straight out of PSUM: 1/rowsum,
                            # broadcast, multiply. (reciprocal_approx_fast
                            # needs an SBUF source: copy the rowsum rows out.)
                            rsA = small_pool.tile([1, 512], F32, tag="rsA")
                            rsB = small_pool.tile([1, 512], F32, tag="rsB")
                            nc.vector.tensor_copy(rsA[:], ctxps[0][64:65, :])
                            nc.vector.tensor_copy(rsB[:], ctxps[1][64:65, :])
                            recipA = small_pool.tile([1, 512], F32, tag="recipA")
                            recipB = small_pool.tile([1, 512], F32, tag="recipB")
                            nc.vector.reciprocal_approx_fast(recipA[:], rsA[:])
                            nc.vector.reciprocal_approx_fast(recipB[:], rsB[:])
                            bcA = small_pool.tile([64, 512], F32, tag="bcA")
                            bcB = small_pool.tile([64, 512], F32, tag="bcB")
                            nc.gpsimd.partition_broadcast(bcA[:], recipA[:])
                            nc.gpsimd.partition_broadcast(bcB[:], recipB[:])
                            nc.vector.scalar_tensor_tensor(
                                ctxT[hp][0:64, qc * 512:(qc + 1) * 512],
                                ctxps[0][0:64, :], 1.0, bcA[:],
                                op0=ALU.mult, op1=ALU.mult)
                            ctmp = tmp_pool.tile([64, 512], BF16, tag="ctmp",
                                                 name="ctmp")
                            nc.vector.scalar_tensor_tensor(
                                ctmp[:], ctxps[1][0:64, :], 1.0, bcB[:],
                                op0=ALU.mult, op1=ALU.mult)
                            nc.sync.dma_start(
                                ctxT[hp][64:128, qc * 512:(qc + 1) * 512],
                                ctmp[:])
                            nc.sync.dma_start(
                                aot_d[hp * 128:(hp + 1) * 128,
                                      qc * 512:(qc + 1) * 512],
                                ctxT[hp][:, qc * 512:(qc + 1) * 512])
                        return _evac

                    for qc in range(QC):
                        for hp in range(3):
                            if hp == 1 and qc > 0:
                                _emit_proj(qc - 1)
                            ring.align(6)
                            ctxps = [ctx_pool.tile([128, 512], F32, tag="ctxps",
                                                   name=f"ctxps{i}")
                                     for i in range(2)]
                            for kt in range(KT):
                                r = kt % 3
                                # flush deferred PV before the ST that may
                                # wait on a ring slot, so the PE has work
                                # queued ahead of the wait
                                while len(pv_defer) > 4:
                                    _emit_pv(pv_defer.pop(0))
                                for ab in range(2):
                                    sts, pos = ring.unit()
                                    ho = ab * 64
                                    nc.tensor.matmul(
                                        sts,
                                        qkT[3 + hp][ho:ho + 64, kt * 128:(kt + 1) * 128],
                                        qkT[hp][ho:ho + 64, qc * 512:(qc + 1) * 512],
                                        start=True, stop=True,
                                        tile_position=(ho, 0))
                                assert pos == 2 * r + 1
                                et1 = exp1_pool.tile([128, 1024], BF16,
                                                     tag="et1", name="et1")
                                if r == 0:
                                    # DVE fast-exp takes every third kt
                                    # (and the last), offloading ScalarE
                                    nc.vector.tensor_scalar(
                                        et1[:].bitcast(I16),
                                        ring.slot(r)[:],
                                        FE_A, FE_B,
                                        op0=ALU.mult, op1=ALU.add)
                                else:
                                    nc.scalar.activation(
                                        et1[:], ring.slot(r)[:], AF.Exp)
                                if kt == 0:
                                    while pending_evac:
                                        pending_evac.pop(0)()
                                batch = {
                                    "work": [
                                        (ctxps[0], et1[:, 0:512], kt, hp * 2),
                                        (ctxps[1], et1[:, 512:1024], kt,
                                         hp * 2 + 1)],
                                    "evac": None}
                                if kt == KT - 1:
                                    batch["evac"] = _make_evac(hp, qc, ctxps)
                                pv_defer.append(batch)
                            # drain: the PV tail runs while the next
                            # iteration's STs wait on nothing
                            while pv_defer:
                                _emit_pv(pv_defer.pop(0))
                    while pending_evac:
                        pending_evac.pop(0)()
                    _emit_proj(QC - 1)


    nc.compile()
    return nc


def _get_nc(repeat=1):
    key = ("nc", repeat)
    if key not in _CACHE:
        _CACHE[key] = _build_nc(repeat)
    return _CACHE[key]


def _prep_inputs(x, W_qkv, b_qkv, W_proj, b_proj):
    x = np.ascontiguousarray(np.asarray(x, dtype=np.float32))
    W_qkv = np.asarray(W_qkv, dtype=np.float32)
    b_qkv = np.asarray(b_qkv, dtype=np.float32)
    W_proj = np.asarray(W_proj, dtype=np.float32)
    b_proj = np.asarray(b_proj, dtype=np.float32)

    bf = ml_dtypes.bfloat16
    in_maps = []
    for c in range(N_CORES):
        b, g = divmod(c, G)
        sl = slice(g * CL, (g + 1) * CL)
        w_q = W_qkv[:, 0:C][:, sl] * SCALE
        w_k = W_qkv[:, C:2 * C][:, sl]
        w_v = np.ascontiguousarray(W_qkv[:, 2 * C:3 * C][:, sl])
        b_q = b_qkv[0:C][sl] * SCALE
        b_k = b_qkv[C:2 * C][sl]
        b_v = b_qkv[2 * C:3 * C][sl]
        w_qk = np.ascontiguousarray(np.concatenate([w_q, w_k], axis=1))
        b_qk = np.ascontiguousarray(
            np.concatenate([b_q, b_k]).reshape(FQK, 128).T)
        w_p = np.ascontiguousarray(W_proj[sl, :])
        bp = b_proj if g == 0 else np.zeros_like(b_proj)
        b_p = np.ascontiguousarray(bp.reshape(C // 128, 128).T)
        in_maps.append({
            "xT": np.ascontiguousarray(x[b].T).astype(bf),
            "w_qk": w_qk.astype(bf),
            "w_v": w_v.astype(bf),
            "w_p": w_p.astype(bf),
            "b_qk": b_qk,
            "b_v": np.ascontiguousarray(b_v[None, :]).astype(bf),
            "b_p": b_p,
            "ones_col": np.ones((1, 128), dtype=bf),
        })
    return in_maps


def run_cores(in_maps, **kw):
    nc = _get_nc()
    return run_bass_kernel_spmd(nc, in_maps, list(range(N_CORES)), **kw)


def gather(results):
    out = np.empty((B, N, C), dtype=np.float32)
    attn_out = np.empty((B, N, C), dtype=np.float32)
    for b in range(B):
        r0 = results[b * G + 0]
        r1 = results[b * G + 1]
        attn_out[b, :, 0:CL] = r0["attn_out_t"].T
        attn_out[b, :, CL:C] = r1["attn_out_t"].T
        out[b] = r0["out_t"].T
        out[b] += r1["out_t"].T
    return out, attn_out


def kernel(x, W_qkv, b_qkv, W_proj, b_proj):
    in_maps = _prep_inputs(x, W_qkv, b_qkv, W_proj, b_proj)
    res = run_cores(in_maps)
    return gather(res.results)



# revision 37
# speedup vs baseline: 1.8382x; 1.0291x over previous
"""Fused multi-head attention + output projection for Trainium2 (Bass/Tile).

Problem: B=4, N=2048, C=768, H=12 heads x D=64.
  qkv = x @ W_qkv + b_qkv ; q,k,v per head ; attn = softmax(q k^T / sqrt(D))
  attn_out = (attn @ v) merged ; out = attn_out @ W_proj + b_proj
  returns (out, attn_out)

Sharding over 8 NeuronCores: core c = (b, g) with b = batch (4), g = head
group (2 groups of 6 heads).  Data-parallel over batch, tensor-parallel over
heads: W_qkv columns / W_proj rows are split per group; the N x N attention
matrix stays core-local.  Host only slices inputs and, on gather, transposes
the (feature-major) outputs and sums the two W_proj partial products per
batch.

Per-core device algorithm (all layouts feature-major "T" = [features, n]):
  xT = transpose(x_b)                       (PE transposes via identity)
  qkT[f, n] = W_qk^T x (+bias, q pre-scaled on host)      fp32r matmuls
  v[n, f] (+bias via ones-row matmul), stored bf16 augmented with a ones
      column per head -> PV matmul also yields softmax row-sums.
  Per head: S^T[k, q] = kT^T qT (no max subtraction needed: |S| <= ~6),
      P^T = exp(S^T) on ScalarE straight out of PSUM (bf16),
      ctx^T[d, q] (+rowsum row) = [v|1]^T @ P^T, normalize by 1/rowsum.
  out^T = W_proj^T ctx^T (+b_proj on group-0 cores only, via zeroed input).

All phases share one 6-bank PSUM "ring" of [128,512] units (plus 2 banks of
PV accumulators), so no PSUM pool boundary serializes phase transitions.
"""

import os
import numpy as np
import ml_dtypes
from contextlib import ExitStack

import concourse.bass as bass
import concourse.tile as tile
import concourse.mybir as mybir
from concourse import bacc
import concourse.bass_utils as _bass_utils
from concourse.bass_utils import run_bass_kernel_spmd

# walrus is invoked with --enable-ldw-opt=false by default, which forces a
# serial LDWEIGHTS before every MATMUL (~250us of PE time for this kernel).
_orig_run_command = _bass_utils.run_command


def _run_command_ldw(argv, **kw):
    argv = ["--enable-ldw-opt=true" if a == "--enable-ldw-opt=false" else a
            for a in argv]
    return _orig_run_command(argv, **kw)


# NOTE: tried --enable-ldw-opt=true: walrus rejects it for fp32/fp32r
# weights ("InstLdweights is not compatible with LDW optimization").
ENABLE_LDW_OPT = bool(os.environ.get("K_LDW_OPT"))
if ENABLE_LDW_OPT and _bass_utils.run_command is _orig_run_command:
    _bass_utils.run_command = _run_command_ldw

N_CORES = 8
B, N, C = 4, 2048, 768
H, D = 12, 64
G = 2                # head groups (tensor-parallel)
HL = H // G          # heads per core
CL = HL * D          # local feature width (384)
SCALE = D ** -0.5
NT = N // 128        # 16 row tiles
CC = C // 128        # 6 contraction chunks
QC = N // 512        # 4 q chunks of 512
KT = N // 128        # 16 k tiles
FQK = 2 * CL // 128  # 6 feature tiles for q|k

F32 = mybir.dt.float32
F32R = mybir.dt.float32r
BF16 = mybir.dt.bfloat16
I16 = mybir.dt.int16

# Schraudolph fast-exp in bf16 bit space: bf16_bits(exp(x)) ~ x*FE_A + FE_B.
# FE_B calibrated on HW (trunc semantics) for zero-mean relative error.
FE_A = 2.0 ** 7 / float(np.log(2.0))
FE_B = 16256.5 - 7.88
AF = mybir.ActivationFunctionType
ALU = mybir.AluOpType

_CACHE = {}


class Ring:
    """Rotating [128, 512] PSUM units across three persistent 2-bank
    slot tiles. Separate tiles keep the tile-level dependency tracking
    per-slot: the ST matmul reusing slot s waits only on that slot's
    exp reader three kts back, not on every in-flight ring access."""

    def __init__(self, slots):
        self.slots = slots          # 3 tiles of [128, 1024]
        self.pos = 0

    def unit(self, width=512):
        p = self.pos % 6
        self.pos += 1
        return self.slots[p // 2][:, (p % 2) * 512:(p % 2) * 512 + width], p

    def slot(self, r):
        return self.slots[r]

    def align(self, m):
        self.pos = ((self.pos + m - 1) // m) * m


def _build_nc(repeat=1):
    nc = bacc.Bacc("TRN2", target_bir_lowering=False, debug=False,
                   num_devices=N_CORES)
    xt_d = nc.dram_tensor("xT", [C, N], BF16, kind="ExternalInput").ap()
    wqk_d = nc.dram_tensor("w_qk", [C, 2 * CL], BF16, kind="ExternalInput").ap()
    wv_d = nc.dram_tensor("w_v", [C, CL], BF16, kind="ExternalInput").ap()
    wp_d = nc.dram_tensor("w_p", [CL, C], BF16, kind="ExternalInput").ap()
    bqk_d = nc.dram_tensor("b_qk", [128, FQK], F32, kind="ExternalInput").ap()
    bv_d = nc.dram_tensor("b_v", [1, CL], BF16, kind="ExternalInput").ap()
    bp_d = nc.dram_tensor("b_p", [128, C // 128], F32, kind="ExternalInput").ap()
    onesc_d = nc.dram_tensor("ones_col", [1, 128], BF16, kind="ExternalInput").ap()
    aot_d = nc.dram_tensor("attn_out_t", [CL, N], BF16, kind="ExternalOutput").ap()
    out_d = nc.dram_tensor("out_t", [C, N], F32R, kind="ExternalOutput").ap()
    DEBUG = bool(os.environ.get("K_DEBUG"))
    if DEBUG:
        dbg_rs = nc.dram_tensor("dbg_rs", [1, 512], F32, kind="ExternalOutput").ap()
        dbg_rA = nc.dram_tensor("dbg_rA", [1, 512], F32, kind="ExternalOutput").ap()
        dbg_bc = nc.dram_tensor("dbg_bc", [64, 512], F32, kind="ExternalOutput").ap()

    with tile.TileContext(nc) as tc:
      for _rep in range(repeat):
        with ExitStack() as top:
            const_pool = top.enter_context(tc.tile_pool(name="const", bufs=1))
            bias_pool = top.enter_context(tc.tile_pool(name="bias", bufs=3))
            qkT_pool = top.enter_context(tc.tile_pool(name="qkT", bufs=FQK))
            vaug_pool = top.enter_context(tc.tile_pool(name="vaug", bufs=NT))
            wp_pool = top.enter_context(tc.tile_pool(name="wp", bufs=3))
            ring_pool = top.enter_context(
                tc.tile_pool(name="ring", bufs=1, space="PSUM"))

            ring = Ring([ring_pool.tile([128, 1024], F32, tag=f"ring{i}",
                                        name=f"ringt{i}") for i in range(3)])

            # x arrives pre-transposed from the host: plain parallel DMA
            # loads instead of the serialized transpose-xbar path.
            xT_pool_o = top.enter_context(tc.tile_pool(name="xT", bufs=CC))
            xT = [xT_pool_o.tile([128, N], BF16, tag="xT", name=f"xTt{i}")
                  for i in range(CC)]

            ones_col = const_pool.tile([1, 128], BF16, tag="ones")
            b_qk = bias_pool.tile([128, FQK], F32, tag="bqk")
            b_p = bias_pool.tile([128, C // 128], F32, tag="bp")
            b_v = bias_pool.tile([1, CL], BF16, tag="bv")

            qkT = [qkT_pool.tile([128, N], BF16, tag="qkT", name=f"qkT{i}")
                   for i in range(FQK)]
            vaug = [vaug_pool.tile([128, HL * 128], BF16, tag="vaug",
                                   name=f"vaug{i}") for i in range(NT)]
            # ------------- Phases A (xT), B (qkT), C (v) -------------
            with ExitStack() as s1:
                wv_pool = s1.enter_context(tc.tile_pool(name="wv", bufs=CC))
                wv = [wv_pool.tile([128, CL], BF16, tag="wv", name=f"wv{cc}")
                      for cc in range(CC)]

                with ExitStack() as s1a:
                    wqk_pool = s1a.enter_context(
                        tc.tile_pool(name="wqk", bufs=CC))
                    wqk = [wqk_pool.tile([128, 2 * CL], BF16, tag="wqk",
                                         name=f"wqk{cc}") for cc in range(CC)]
                    # DMA issue order = need order: the B-phase matmuls
                    # consume (wqk[cc], xT[cc]) pairs first.
                    for cc in range(CC):
                        nc.sync.dma_start(wqk[cc][:],
                                          wqk_d[cc * 128:(cc + 1) * 128, :])
                        nc.sync.dma_start(xT[cc][:],
                                          xt_d[cc * 128:(cc + 1) * 128, :])
                    nc.sync.dma_start(b_qk[:], bqk_d[:])
                    for cc in range(CC):
                        nc.sync.dma_start(wv[cc][:],
                                          wv_d[cc * 128:(cc + 1) * 128, :])
                    nc.sync.dma_start(ones_col[:], onesc_d[:])
                    nc.sync.dma_start(b_v[:], bv_d[:])
                    nc.sync.dma_start(b_p[:], bp_d[:])
                    wp = []
                    for i in range(3):
                        t = wp_pool.tile([128, C], BF16, tag="wp", name=f"wp{i}")
                        nc.sync.dma_start(t[:], wp_d[i * 128:(i + 1) * 128, :])
                        wp.append(t)

                    # B: qkT = W_qk^T @ x^T (+ per-partition bias on evac)
                    for ft in range(FQK):
                        for qc in range(QC):
                            ps, _ = ring.unit()
                            for cc in range(CC):
                                nc.tensor.matmul(
                                    ps[:], wqk[cc][:, ft * 128:(ft + 1) * 128],
                                    xT[cc][:, qc * 512:(qc + 1) * 512],
                                    start=(cc == 0), stop=(cc == CC - 1))
                            if qc % 2 == 0:
                                nc.vector.tensor_scalar_add(
                                    qkT[ft][:, qc * 512:(qc + 1) * 512], ps[:],
                                    b_qk[:, ft:ft + 1])
                            else:
                                nc.scalar.activation(
                                    qkT[ft][:, qc * 512:(qc + 1) * 512], ps[:],
                                    AF.Identity, bias=b_qk[:, ft:ft + 1])

                # C: v natural (+bias via ones-row), ones col per head
                for nt in range(NT):
                    ps, _ = ring.unit(width=CL)
                    for cc in range(CC):
                        nc.tensor.matmul(
                            ps[:], xT[cc][:, nt * 128:(nt + 1) * 128], wv[cc][:],
                            start=(cc == 0), stop=False)
                    nc.tensor.matmul(ps[:], ones_col[:], b_v[:],
                                     start=False, stop=True)
                    # vaug is padded to 128 cols per head (FWL-eligible
                    # ldweights): [v(64) | ones(1) | zeros(63)]
                    va3 = vaug[nt][:].rearrange("p (h e) -> p h e", e=128)
                    if nt % 2 == 0:
                        nc.vector.tensor_copy(
                            va3[:, :, 0:64],
                            ps[:].rearrange("p (h e) -> p h e", e=64))
                    else:
                        nc.scalar.activation(
                            va3[:, :, 0:64],
                            ps[:].rearrange("p (h e) -> p h e", e=64),
                            AF.Identity)
                    nc.vector.memset(va3[:, :, 64:65], 1.0)
                    nc.gpsimd.memset(va3[:, :, 65:128], 0.0)

            # ---------------- Phases D (attention) + E (proj) ----------------
            # qc-major so the output projection for a q-chunk can overlap the
            # next chunk's attention. Inner loop is pipelined per kt-triplet:
            # ring slots of 2 units per kt (pos 0-1 / 2-3 / 4-5); exp for kts
            # 0,1 of a triplet is one ScalarE ACTIVATE over ring[0:2048], kt 2
            # goes to VectorE via a Schraudolph fast-exp (int16 bit trick ->
            # bf16), offloading 1/3 of the exp work so ScalarE never gates the
            # PE. PV matmuls trail by one triplet.
            with ExitStack() as s23:
                ctxT_pool = s23.enter_context(tc.tile_pool(name="ctxT", bufs=3))
                ctxT = [ctxT_pool.tile([128, N], BF16, tag="ctxT",
                                       name=f"ctxT{i}") for i in range(3)]

                with ExitStack() as s2, ExitStack() as s3:
                    ctx_pool = s2.enter_context(
                        tc.tile_pool(name="ctxps", bufs=2, space="PSUM"))
                    exp1_pool = s2.enter_context(tc.tile_pool(name="et1", bufs=10))
                    small_pool = s2.enter_context(tc.tile_pool(name="small", bufs=4))
                    tmp_pool = s2.enter_context(tc.tile_pool(name="ctmp", bufs=2))
                    out_pool = s3.enter_context(tc.tile_pool(name="outT", bufs=4))

                    def _emit_proj(qcp):
                        # E: out^T = W_proj^T ctx^T (+bias) for q chunk qcp
                        for of in range(C // 128):
                            ps, _ = ring.unit()
                            for c2 in range(3):
                                nc.tensor.matmul(
                                    ps[:], wp[c2][:, of * 128:(of + 1) * 128],
                                    ctxT[c2][:, qcp * 512:(qcp + 1) * 512],
                                    start=(c2 == 0), stop=(c2 == 2))
                            ot = out_pool.tile([128, 512], F32R, tag="outT",
                                               name="ot")
                            if of % 2 == 0:
                                nc.vector.tensor_scalar_add(ot[:], ps[:],
                                                            b_p[:, of:of + 1])
                            else:
                                nc.scalar.activation(ot[:], ps[:], AF.Identity,
                                                     bias=b_p[:, of:of + 1])
                            nc.sync.dma_start(
                                out_d[of * 128:(of + 1) * 128,
                                      qcp * 512:(qcp + 1) * 512], ot[:])

                    pv_defer = []   # deferred PV work: (kk, [et_ap0, et_ap1])
                    pending_norm = []  # normalize stage, deferred into the
                                       # next iteration (kt==2) so the stt
                                       # multiplies don't head-of-line block
                                       # the Vector FIFO behind the GpSimd
                                       # broadcast at the boundary

                    def _emit_pv(batch):
                        for ctxps, et_ap, kk, lh in batch["work"]:
                            nc.tensor.matmul(
                                ctxps[:],
                                vaug[kk][:, lh * 128:(lh + 1) * 128],
                                et_ap,
                                start=(kk == 0), stop=(kk == KT - 1))
                        if batch["evac"] is not None:
                            batch["evac"]()

                    def _make_evac(hp, qc, ctxps):
                        def _evac():
                            # stage 1 (at drain): rowsum rows out of PSUM,
                            # reciprocal, partition-broadcast
                            rsA = small_pool.tile([1, 512], F32, tag="rsA")
                            rsB = small_pool.tile([1, 512], F32, tag="rsB")
                            nc.vector.tensor_copy(rsA[:], ctxps[0][64:65, :])
                            nc.vector.tensor_copy(rsB[:], ctxps[1][64:65, :])
                            recipA = small_pool.tile([1, 512], F32, tag="recipA")
                            recipB = small_pool.tile([1, 512], F32, tag="recipB")
                            nc.vector.reciprocal_approx_fast(recipA[:], rsA[:])
                            nc.vector.reciprocal_approx_fast(recipB[:], rsB[:])
                            bcA = small_pool.tile([64, 512], F32, tag="bcA")
                            bcB = small_pool.tile([64, 512], F32, tag="bcB")
                            nc.gpsimd.partition_broadcast(bcA[:], recipA[:])
                            nc.gpsimd.partition_broadcast(bcB[:], recipB[:])
                            pending_norm.append(
                                lambda: _emit_norm(hp, qc, ctxps, bcA, bcB))
                        return _evac

                    def _emit_norm(hp, qc, ctxps, bcA, bcB):
                            nc.vector.scalar_tensor_tensor(
                                ctxT[hp][0:64, qc * 512:(qc + 1) * 512],
                                ctxps[0][0:64, :], 1.0, bcA[:],
                                op0=ALU.mult, op1=ALU.mult)
                            ctmp = tmp_pool.tile([64, 512], BF16, tag="ctmp",
                                                 name="ctmp")
                            nc.vector.scalar_tensor_tensor(
                                ctmp[:], ctxps[1][0:64, :], 1.0, bcB[:],
                                op0=ALU.mult, op1=ALU.mult)
                            nc.sync.dma_start(
                                ctxT[hp][64:128, qc * 512:(qc + 1) * 512],
                                ctmp[:])
                            nc.sync.dma_start(
                                aot_d[hp * 128:(hp + 1) * 128,
                                      qc * 512:(qc + 1) * 512],
                                ctxT[hp][:, qc * 512:(qc + 1) * 512])

                    for qc in range(QC):
                        for hp in range(3):
                            if hp == 1 and qc > 0:
                                _emit_proj(qc - 1)
                            ring.align(6)
                            ctxps = [ctx_pool.tile([128, 512], F32, tag="ctxps",
                                                   name=f"ctxps{i}")
                                     for i in range(2)]
                            for kt in range(KT):
                                r = kt % 3
                                # flush deferred PV before the ST that may
                                # wait on a ring slot, so the PE has work
                                # queued ahead of the wait
                                while len(pv_defer) > 4:
                                    _emit_pv(pv_defer.pop(0))
                                for ab in range(2):
                                    sts, pos = ring.unit()
                                    ho = ab * 64
                                    nc.tensor.matmul(
                                        sts,
                                        qkT[3 + hp][ho:ho + 64, kt * 128:(kt + 1) * 128],
                                        qkT[hp][ho:ho + 64, qc * 512:(qc + 1) * 512],
                                        start=True, stop=True,
                                        tile_position=(ho, 0))
                                assert pos == 2 * r + 1
                                et1 = exp1_pool.tile([128, 1024], BF16,
                                                     tag="et1", name="et1")
                                if r == 0:
                                    # DVE fast-exp takes every third kt
                                    # (and the last), offloading ScalarE
                                    nc.vector.tensor_scalar(
                                        et1[:].bitcast(I16),
                                        ring.slot(r)[:],
                                        FE_A, FE_B,
                                        op0=ALU.mult, op1=ALU.add)
                                else:
                                    nc.scalar.activation(
                                        et1[:], ring.slot(r)[:], AF.Exp)
                                if kt == 2:
                                    while pending_norm:
                                        pending_norm.pop(0)()
                                batch = {
                                    "work": [
                                        (ctxps[0], et1[:, 0:512], kt, hp * 2),
                                        (ctxps[1], et1[:, 512:1024], kt,
                                         hp * 2 + 1)],
                                    "evac": None}
                                if kt == KT - 1:
                                    batch["evac"] = _make_evac(hp, qc, ctxps)
                                pv_defer.append(batch)
                            # drain: the PV tail runs while the next
                            # iteration's STs wait on nothing
                            while pv_defer:
                                _emit_pv(pv_defer.pop(0))
                    while pending_norm:
                        pending_norm.pop(0)()
                    _emit_proj(QC - 1)


    nc.compile()
    return nc


def _get_nc(repeat=1):
    key = ("nc", repeat)
    if key not in _CACHE:
        _CACHE[key] = _build_nc(repeat)
    return _CACHE[key]


def _prep_inputs(x, W_qkv, b_qkv, W_proj, b_proj):
    x = np.ascontiguousarray(np.asarray(x, dtype=np.float32))
    W_qkv = np.asarray(W_qkv, dtype=np.float32)
    b_qkv = np.asarray(b_qkv, dtype=np.float32)
    W_proj = np.asarray(W_proj, dtype=np.float32)
    b_proj = np.asarray(b_proj, dtype=np.float32)

    bf = ml_dtypes.bfloat16
    in_maps = []
    for c in range(N_CORES):
        b, g = divmod(c, G)
        sl = slice(g * CL, (g + 1) * CL)
        w_q = W_qkv[:, 0:C][:, sl] * SCALE
        w_k = W_qkv[:, C:2 * C][:, sl]
        w_v = np.ascontiguousarray(W_qkv[:, 2 * C:3 * C][:, sl])
        b_q = b_qkv[0:C][sl] * SCALE
        b_k = b_qkv[C:2 * C][sl]
        b_v = b_qkv[2 * C:3 * C][sl]
        w_qk = np.ascontiguousarray(np.concatenate([w_q, w_k], axis=1))
        b_qk = np.ascontiguousarray(
            np.concatenate([b_q, b_k]).reshape(FQK, 128).T)
        w_p = np.ascontiguousarray(W_proj[sl, :])
        bp = b_proj if g == 0 else np.zeros_like(b_proj)
        b_p = np.ascontiguousarray(bp.reshape(C // 128, 128).T)
        in_maps.append({
            "xT": np.ascontiguousarray(x[b].T).astype(bf),
            "w_qk": w_qk.astype(bf),
            "w_v": w_v.astype(bf),
            "w_p": w_p.astype(bf),
            "b_qk": b_qk,
            "b_v": np.ascontiguousarray(b_v[None, :]).astype(bf),
            "b_p": b_p,
            "ones_col": np.ones((1, 128), dtype=bf),
        })
    return in_maps


def run_cores(in_maps, **kw):
    nc = _get_nc()
    return run_bass_kernel_spmd(nc, in_maps, list(range(N_CORES)), **kw)


def gather(results):
    out = np.empty((B, N, C), dtype=np.float32)
    attn_out = np.empty((B, N, C), dtype=np.float32)
    for b in range(B):
        r0 = results[b * G + 0]
        r1 = results[b * G + 1]
        attn_out[b, :, 0:CL] = r0["attn_out_t"].T
        attn_out[b, :, CL:C] = r1["attn_out_t"].T
        out[b] = r0["out_t"].T
        out[b] += r1["out_t"].T
    return out, attn_out


def kernel(x, W_qkv, b_qkv, W_proj, b_proj):
    in_maps = _prep_inputs(x, W_qkv, b_qkv, W_proj, b_proj)
    res = run_cores(in_maps)
    return gather(res.results)



# revision 41
# speedup vs baseline: 1.9147x; 1.0416x over previous
"""Fused multi-head attention + output projection for Trainium2 (Bass/Tile).

Problem: B=4, N=2048, C=768, H=12 heads x D=64.
  qkv = x @ W_qkv + b_qkv ; q,k,v per head ; attn = softmax(q k^T / sqrt(D))
  attn_out = (attn @ v) merged ; out = attn_out @ W_proj + b_proj
  returns (out, attn_out)

Sharding over 8 NeuronCores: core c = (b, g) with b = batch (4), g = head
group (2 groups of 6 heads).  Data-parallel over batch, tensor-parallel over
heads: W_qkv columns / W_proj rows are split per group; the N x N attention
matrix stays core-local.  Host only slices inputs and, on gather, transposes
the (feature-major) outputs and sums the two W_proj partial products per
batch.

Per-core device algorithm (all layouts feature-major "T" = [features, n]):
  xT = transpose(x_b)                       (PE transposes via identity)
  qkT[f, n] = W_qk^T x (+bias, q pre-scaled on host)      fp32r matmuls
  v[n, f] (+bias via ones-row matmul), stored bf16 augmented with a ones
      column per head -> PV matmul also yields softmax row-sums.
  Per head: S^T[k, q] = kT^T qT (no max subtraction needed: |S| <= ~6),
      P^T = exp(S^T) on ScalarE straight out of PSUM (bf16),
      ctx^T[d, q] (+rowsum row) = [v|1]^T @ P^T, normalize by 1/rowsum.
  out^T = W_proj^T ctx^T (+b_proj on group-0 cores only, via zeroed input).

All phases share one 6-bank PSUM "ring" of [128,512] units (plus 2 banks of
PV accumulators), so no PSUM pool boundary serializes phase transitions.
"""

import os
import numpy as np
import ml_dtypes
from contextlib import ExitStack

import concourse.bass as bass
import concourse.tile as tile
import concourse.mybir as mybir
from concourse import bacc
import concourse.bass_utils as _bass_utils
from concourse.bass_utils import run_bass_kernel_spmd

# walrus is invoked with --enable-ldw-opt=false by default, which forces a
# serial LDWEIGHTS before every MATMUL (~250us of PE time for this kernel).
_orig_run_command = _bass_utils.run_command


def _run_command_ldw(argv, **kw):
    argv = ["--enable-ldw-opt=true" if a == "--enable-ldw-opt=false" else a
            for a in argv]
    return _orig_run_command(argv, **kw)


# NOTE: tried --enable-ldw-opt=true: walrus rejects it for fp32/fp32r
# weights ("InstLdweights is not compatible with LDW optimization").
ENABLE_LDW_OPT = bool(os.environ.get("K_LDW_OPT"))
if ENABLE_LDW_OPT and _bass_utils.run_command is _orig_run_command:
    _bass_utils.run_command = _run_command_ldw

N_CORES = 8
B, N, C = 4, 2048, 768
H, D = 12, 64
G = 2                # head groups (tensor-parallel)
HL = H // G          # heads per core
CL = HL * D          # local feature width (384)
SCALE = D ** -0.5
NT = N // 128        # 16 row tiles
CC = C // 128        # 6 contraction chunks
QC = N // 512        # 4 q chunks of 512
KT = N // 128        # 16 k tiles
FQK = 2 * CL // 128  # 6 feature tiles for q|k

F32 = mybir.dt.float32
F32R = mybir.dt.float32r
BF16 = mybir.dt.bfloat16
I16 = mybir.dt.int16

# Schraudolph fast-exp in bf16 bit space: bf16_bits(exp(x)) ~ x*FE_A + FE_B.
# FE_B calibrated on HW (trunc semantics) for zero-mean relative error.
FE_A = 2.0 ** 7 / float(np.log(2.0))
FE_B = 16256.5 - 7.88
AF = mybir.ActivationFunctionType
ALU = mybir.AluOpType

_CACHE = {}


class Ring:
    """Rotating [128, 512] PSUM units across three persistent 2-bank
    slot tiles. Separate tiles keep the tile-level dependency tracking
    per-slot: the ST matmul reusing slot s waits only on that slot's
    exp reader three kts back, not on every in-flight ring access."""

    def __init__(self, slots):
        self.slots = slots          # 3 tiles of [128, 1024]
        self.pos = 0

    def unit(self, width=512):
        p = self.pos % 6
        self.pos += 1
        return self.slots[p // 2][:, (p % 2) * 512:(p % 2) * 512 + width], p

    def slot(self, r):
        return self.slots[r]

    def align(self, m):
        self.pos = ((self.pos + m - 1) // m) * m


def _build_nc(repeat=1):
    nc = bacc.Bacc("TRN2", target_bir_lowering=False, debug=False,
                   num_devices=N_CORES)
    xt_d = nc.dram_tensor("xT", [C, N], BF16, kind="ExternalInput").ap()
    wqk_d = nc.dram_tensor("w_qk", [C, 2 * CL], BF16, kind="ExternalInput").ap()
    wv_d = nc.dram_tensor("w_v", [C, CL], BF16, kind="ExternalInput").ap()
    wp_d = nc.dram_tensor("w_p", [CL, C], BF16, kind="ExternalInput").ap()
    bqk_d = nc.dram_tensor("b_qk", [128, FQK], F32, kind="ExternalInput").ap()
    bv_d = nc.dram_tensor("b_v", [1, CL], BF16, kind="ExternalInput").ap()
    bp_d = nc.dram_tensor("b_p", [128, C // 128], F32, kind="ExternalInput").ap()
    onesc_d = nc.dram_tensor("ones_col", [1, 128], BF16, kind="ExternalInput").ap()
    aot_d = nc.dram_tensor("attn_out_t", [CL, N], BF16, kind="ExternalOutput").ap()
    out_d = nc.dram_tensor("out_t", [C, N], F32R, kind="ExternalOutput").ap()
    DEBUG = bool(os.environ.get("K_DEBUG"))
    if DEBUG:
        dbg_rs = nc.dram_tensor("dbg_rs", [1, 512], F32, kind="ExternalOutput").ap()
        dbg_rA = nc.dram_tensor("dbg_rA", [1, 512], F32, kind="ExternalOutput").ap()
        dbg_bc = nc.dram_tensor("dbg_bc", [64, 512], F32, kind="ExternalOutput").ap()

    with tile.TileContext(nc) as tc:
      for _rep in range(repeat):
        with ExitStack() as top:
            const_pool = top.enter_context(tc.tile_pool(name="const", bufs=1))
            bias_pool = top.enter_context(tc.tile_pool(name="bias", bufs=3))
            qkT_pool = top.enter_context(tc.tile_pool(name="qkT", bufs=FQK))
            vaug_pool = top.enter_context(tc.tile_pool(name="vaug", bufs=NT))
            wp_pool = top.enter_context(tc.tile_pool(name="wp", bufs=3))
            ring_pool = top.enter_context(
                tc.tile_pool(name="ring", bufs=1, space="PSUM"))

            ring = Ring([ring_pool.tile([128, 1024], F32, tag=f"ring{i}",
                                        name=f"ringt{i}") for i in range(3)])

            # x arrives pre-transposed from the host: plain parallel DMA
            # loads instead of the serialized transpose-xbar path.
            xT_pool_o = top.enter_context(tc.tile_pool(name="xT", bufs=CC))
            xT = [xT_pool_o.tile([128, N], BF16, tag="xT", name=f"xTt{i}")
                  for i in range(CC)]

            ones_col = const_pool.tile([1, 128], BF16, tag="ones")
            b_qk = bias_pool.tile([128, FQK], F32, tag="bqk")
            b_p = bias_pool.tile([128, C // 128], F32, tag="bp")
            b_v = bias_pool.tile([1, CL], BF16, tag="bv")

            qkT = [qkT_pool.tile([128, N], BF16, tag="qkT", name=f"qkT{i}")
                   for i in range(FQK)]
            vaug = [vaug_pool.tile([128, HL * 128], BF16, tag="vaug",
                                   name=f"vaug{i}") for i in range(NT)]
            # ------------- Phases A (xT), B (qkT), C (v) -------------
            with ExitStack() as s1:
                wv_pool = s1.enter_context(tc.tile_pool(name="wv", bufs=CC))
                wv = [wv_pool.tile([128, CL], BF16, tag="wv", name=f"wv{cc}")
                      for cc in range(CC)]

                with ExitStack() as s1a:
                    wqk_pool = s1a.enter_context(
                        tc.tile_pool(name="wqk", bufs=CC))
                    wqk = [wqk_pool.tile([128, 2 * CL], BF16, tag="wqk",
                                         name=f"wqk{cc}") for cc in range(CC)]
                    # DMA issue order = need order: the B-phase matmuls
                    # consume (wqk[cc], xT[cc]) pairs first.
                    for cc in range(CC):
                        nc.sync.dma_start(wqk[cc][:],
                                          wqk_d[cc * 128:(cc + 1) * 128, :])
                        nc.sync.dma_start(xT[cc][:],
                                          xt_d[cc * 128:(cc + 1) * 128, :])
                    nc.sync.dma_start(b_qk[:], bqk_d[:])
                    for cc in range(CC):
                        nc.sync.dma_start(wv[cc][:],
                                          wv_d[cc * 128:(cc + 1) * 128, :])
                    nc.sync.dma_start(ones_col[:], onesc_d[:])
                    nc.sync.dma_start(b_v[:], bv_d[:])
                    nc.sync.dma_start(b_p[:], bp_d[:])
                    wp = []
                    for i in range(3):
                        t = wp_pool.tile([128, C], BF16, tag="wp", name=f"wp{i}")
                        nc.sync.dma_start(t[:], wp_d[i * 128:(i + 1) * 128, :])
                        wp.append(t)

                    # B: qkT = W_qk^T @ x^T (+ per-partition bias on evac)
                    for ft in range(FQK):
                        for qc in range(QC):
                            ps, _ = ring.unit()
                            for cc in range(CC):
                                nc.tensor.matmul(
                                    ps[:], wqk[cc][:, ft * 128:(ft + 1) * 128],
                                    xT[cc][:, qc * 512:(qc + 1) * 512],
                                    start=(cc == 0), stop=(cc == CC - 1))
                            if qc % 2 == 0:
                                nc.vector.tensor_scalar_add(
                                    qkT[ft][:, qc * 512:(qc + 1) * 512], ps[:],
                                    b_qk[:, ft:ft + 1])
                            else:
                                nc.scalar.activation(
                                    qkT[ft][:, qc * 512:(qc + 1) * 512], ps[:],
                                    AF.Identity, bias=b_qk[:, ft:ft + 1])

                # C: v natural (+bias via ones-row), ones col per head
                for nt in range(NT):
                    ps, _ = ring.unit(width=CL)
                    for cc in range(CC):
                        nc.tensor.matmul(
                            ps[:], xT[cc][:, nt * 128:(nt + 1) * 128], wv[cc][:],
                            start=(cc == 0), stop=False)
                    nc.tensor.matmul(ps[:], ones_col[:], b_v[:],
                                     start=False, stop=True)
                    # vaug is padded to 128 cols per head (FWL-eligible
                    # ldweights): [v(64) | ones(1) | zeros(63)]
                    va3 = vaug[nt][:].rearrange("p (h e) -> p h e", e=128)
                    if nt % 2 == 0:
                        nc.vector.tensor_copy(
                            va3[:, :, 0:64],
                            ps[:].rearrange("p (h e) -> p h e", e=64))
                    else:
                        nc.scalar.activation(
                            va3[:, :, 0:64],
                            ps[:].rearrange("p (h e) -> p h e", e=64),
                            AF.Identity)
                    nc.vector.memset(va3[:, :, 64:65], 1.0)
                    nc.gpsimd.memset(va3[:, :, 65:128], 0.0)

            # ---------------- Phases D (attention) + E (proj) ----------------
            # qc-major so the output projection for a q-chunk can overlap the
            # next chunk's attention. Inner loop is pipelined per kt-triplet:
            # ring slots of 2 units per kt (pos 0-1 / 2-3 / 4-5); exp for kts
            # 0,1 of a triplet is one ScalarE ACTIVATE over ring[0:2048], kt 2
            # goes to VectorE via a Schraudolph fast-exp (int16 bit trick ->
            # bf16), offloading 1/3 of the exp work so ScalarE never gates the
            # PE. PV matmuls trail by one triplet.
            with ExitStack() as s23:
                ctxT_pool = s23.enter_context(tc.tile_pool(name="ctxT", bufs=3))
                ctxT = [ctxT_pool.tile([128, N], BF16, tag="ctxT",
                                       name=f"ctxT{i}") for i in range(3)]

                with ExitStack() as s2, ExitStack() as s3:
                    ctx_pool = s2.enter_context(
                        tc.tile_pool(name="ctxps", bufs=2, space="PSUM"))
                    exp1_pool = s2.enter_context(tc.tile_pool(name="et1", bufs=10))
                    small_pool = s2.enter_context(tc.tile_pool(name="small", bufs=4))
                    tmp_pool = s2.enter_context(tc.tile_pool(name="ctmp", bufs=2))
                    out_pool = s3.enter_context(tc.tile_pool(name="outT", bufs=4))

                    def _emit_proj(qcp):
                        # E: out^T = W_proj^T ctx^T (+bias) for q chunk qcp
                        for of in range(C // 128):
                            ps, _ = ring.unit()
                            for c2 in range(3):
                                nc.tensor.matmul(
                                    ps[:], wp[c2][:, of * 128:(of + 1) * 128],
                                    ctxT[c2][:, qcp * 512:(qcp + 1) * 512],
                                    start=(c2 == 0), stop=(c2 == 2))
                            ot = out_pool.tile([128, 512], F32R, tag="outT",
                                               name="ot")
                            if of % 2 == 0:
                                nc.vector.tensor_scalar_add(ot[:], ps[:],
                                                            b_p[:, of:of + 1])
                            else:
                                nc.scalar.activation(ot[:], ps[:], AF.Identity,
                                                     bias=b_p[:, of:of + 1])
                            nc.sync.dma_start(
                                out_d[of * 128:(of + 1) * 128,
                                      qcp * 512:(qcp + 1) * 512], ot[:])

                    pv_defer = []   # deferred PV work: (kk, [et_ap0, et_ap1])
                    # The softmax-normalize chain is staged across the next
                    # iteration so none of it ever sits ahead of a fast-exp
                    # in the Vector FIFO: rowsum copies ride ScalarE at drain
                    # time, reciprocal+broadcast flush at kt==1, and the
                    # final multiplies flush at kt==5.
                    pending_recip = []
                    pending_norm = []

                    def _emit_pv(batch):
                        for ctxps, et_ap, kk, lh in batch["work"]:
                            nc.tensor.matmul(
                                ctxps[:],
                                vaug[kk][:, lh * 128:(lh + 1) * 128],
                                et_ap,
                                start=(kk == 0), stop=(kk == KT - 1))
                        if batch["evac"] is not None:
                            batch["evac"]()

                    def _make_evac(hp, qc, ctxps):
                        def _evac():
                            # stage 1 (at drain): rowsum rows out of PSUM on
                            # ScalarE (keeps the Vector FIFO clear)
                            rsA = small_pool.tile([1, 512], F32, tag="rsA")
                            rsB = small_pool.tile([1, 512], F32, tag="rsB")
                            nc.scalar.activation(rsA[:], ctxps[0][64:65, :],
                                                 AF.Identity)
                            nc.scalar.activation(rsB[:], ctxps[1][64:65, :],
                                                 AF.Identity)
                            pending_recip.append(
                                lambda: _emit_recip(hp, qc, ctxps, rsA, rsB))
                        return _evac

                    def _emit_recip(hp, qc, ctxps, rsA, rsB):
                        recipA = small_pool.tile([1, 512], F32, tag="recipA")
                        recipB = small_pool.tile([1, 512], F32, tag="recipB")
                        nc.vector.reciprocal_approx_fast(recipA[:], rsA[:])
                        nc.vector.reciprocal_approx_fast(recipB[:], rsB[:])
                        bcA = small_pool.tile([64, 512], F32, tag="bcA")
                        bcB = small_pool.tile([64, 512], F32, tag="bcB")
                        nc.gpsimd.partition_broadcast(bcA[:], recipA[:])
                        nc.gpsimd.partition_broadcast(bcB[:], recipB[:])
                        pending_norm.append(
                            lambda: _emit_norm(hp, qc, ctxps, bcA, bcB))

                    def _emit_norm(hp, qc, ctxps, bcA, bcB):
                            nc.vector.scalar_tensor_tensor(
                                ctxT[hp][0:64, qc * 512:(qc + 1) * 512],
                                ctxps[0][0:64, :], 1.0, bcA[:],
                                op0=ALU.mult, op1=ALU.mult)
                            ctmp = tmp_pool.tile([64, 512], BF16, tag="ctmp",
                                                 name="ctmp")
                            nc.vector.scalar_tensor_tensor(
                                ctmp[:], ctxps[1][0:64, :], 1.0, bcB[:],
                                op0=ALU.mult, op1=ALU.mult)
                            nc.sync.dma_start(
                                ctxT[hp][64:128, qc * 512:(qc + 1) * 512],
                                ctmp[:])
                            nc.sync.dma_start(
                                aot_d[hp * 128:(hp + 1) * 128,
                                      qc * 512:(qc + 1) * 512],
                                ctxT[hp][:, qc * 512:(qc + 1) * 512])

                    for qc in range(QC):
                        for hp in range(3):
                            if hp == 1 and qc > 0:
                                _emit_proj(qc - 1)
                            ring.align(6)
                            ctxps = [ctx_pool.tile([128, 512], F32, tag="ctxps",
                                                   name=f"ctxps{i}")
                                     for i in range(2)]
                            for kt in range(KT):
                                r = kt % 3
                                # flush deferred PV before the ST that may
                                # wait on a ring slot, so the PE has work
                                # queued ahead of the wait
                                while len(pv_defer) > 6:
                                    _emit_pv(pv_defer.pop(0))
                                for ab in range(2):
                                    sts, pos = ring.unit()
                                    ho = ab * 64
                                    nc.tensor.matmul(
                                        sts,
                                        qkT[3 + hp][ho:ho + 64, kt * 128:(kt + 1) * 128],
                                        qkT[hp][ho:ho + 64, qc * 512:(qc + 1) * 512],
                                        start=True, stop=True,
                                        tile_position=(ho, 0))
                                assert pos == 2 * r + 1
                                et1 = exp1_pool.tile([128, 1024], BF16,
                                                     tag="et1", name="et1")
                                if r == 0:
                                    # DVE fast-exp takes every third kt
                                    # (and the last), offloading ScalarE
                                    nc.vector.tensor_scalar(
                                        et1[:].bitcast(I16),
                                        ring.slot(r)[:],
                                        FE_A, FE_B,
                                        op0=ALU.mult, op1=ALU.add)
                                else:
                                    nc.scalar.activation(
                                        et1[:], ring.slot(r)[:], AF.Exp)
                                if kt == 1:
                                    while pending_recip:
                                        pending_recip.pop(0)()
                                elif kt == 5:
                                    while pending_norm:
                                        pending_norm.pop(0)()
                                batch = {
                                    "work": [
                                        (ctxps[0], et1[:, 0:512], kt, hp * 2),
                                        (ctxps[1], et1[:, 512:1024], kt,
                                         hp * 2 + 1)],
                                    "evac": None}
                                if kt == KT - 1:
                                    batch["evac"] = _make_evac(hp, qc, ctxps)
                                pv_defer.append(batch)
                            # drain: the PV tail runs while the next
                            # iteration's STs wait on nothing
                            while pv_defer:
                                _emit_pv(pv_defer.pop(0))
                    while pending_recip:
                        pending_recip.pop(0)()
                    while pending_norm:
                        pending_norm.pop(0)()
                    _emit_proj(QC - 1)


    nc.compile()
    return nc


def _get_nc(repeat=1):
    key = ("nc", repeat)
    if key not in _CACHE:
        _CACHE[key] = _build_nc(repeat)
    return _CACHE[key]


def _prep_inputs(x, W_qkv, b_qkv, W_proj, b_proj):
    x = np.ascontiguousarray(np.asarray(x, dtype=np.float32))
    W_qkv = np.asarray(W_qkv, dtype=np.float32)
    b_qkv = np.asarray(b_qkv, dtype=np.float32)
    W_proj = np.asarray(W_proj, dtype=np.float32)
    b_proj = np.asarray(b_proj, dtype=np.float32)

    bf = ml_dtypes.bfloat16
    in_maps = []
    for c in range(N_CORES):
        b, g = divmod(c, G)
        sl = slice(g * CL, (g + 1) * CL)
        w_q = W_qkv[:, 0:C][:, sl] * SCALE
        w_k = W_qkv[:, C:2 * C][:, sl]
        w_v = np.ascontiguousarray(W_qkv[:, 2 * C:3 * C][:, sl])
        b_q = b_qkv[0:C][sl] * SCALE
        b_k = b_qkv[C:2 * C][sl]
        b_v = b_qkv[2 * C:3 * C][sl]
        w_qk = np.ascontiguousarray(np.concatenate([w_q, w_k], axis=1))
        b_qk = np.ascontiguousarray(
            np.concatenate([b_q, b_k]).reshape(FQK, 128).T)
        w_p = np.ascontiguousarray(W_proj[sl, :])
        bp = b_proj if g == 0 else np.zeros_like(b_proj)
        b_p = np.ascontiguousarray(bp.reshape(C // 128, 128).T)
        in_maps.append({
            "xT": np.ascontiguousarray(x[b].T).astype(bf),
            "w_qk": w_qk.astype(bf),
            "w_v": w_v.astype(bf),
            "w_p": w_p.astype(bf),
            "b_qk": b_qk,
            "b_v": np.ascontiguousarray(b_v[None, :]).astype(bf),
            "b_p": b_p,
            "ones_col": np.ones((1, 128), dtype=bf),
        })
    return in_maps


def run_cores(in_maps, **kw):
    nc = _get_nc()
    return run_bass_kernel_spmd(nc, in_maps, list(range(N_CORES)), **kw)


def gather(results):
    out = np.empty((B, N, C), dtype=np.float32)
    attn_out = np.empty((B, N, C), dtype=np.float32)
    for b in range(B):
        r0 = results[b * G + 0]
        r1 = results[b * G + 1]
        attn_out[b, :, 0:CL] = r0["attn_out_t"].T
        attn_out[b, :, CL:C] = r1["attn_out_t"].T
        out[b] = r0["out_t"].T
        out[b] += r1["out_t"].T
    return out, attn_out


def kernel(x, W_qkv, b_qkv, W_proj, b_proj):
    in_maps = _prep_inputs(x, W_qkv, b_qkv, W_proj, b_proj)
    res = run_cores(in_maps)
    return gather(res.results)

